# revision 15
# baseline (speedup 1.0000x reference)
"""Trainium2 Bass kernel for nn_MultiHeadedAttention_25984552141341.

Computation (reference):
    q/k/v = (x @ W + b) split into 8 heads of 64
    scores = q @ k^T / 8
    scores += sf_net(scores)   (SoftmaxResNet over the key dim, 71)
    p = softmax(scores, axis=key)
    out = (p @ v merged) @ Wo + bo

Sharding: batch (512) split across 8 NeuronCores, 64 batches each.
All weights replicated. Each core runs an identical Bass program (SPMD).

Device-side layout strategy (per core):
  * activations live feature-on-partition / token-on-free ("transposed"
    layout, xT = [D, B*L]); the host passes query/key/value pre-transposed
    AND pre-cast to bf16 (the matmuls run in bf16 anyway), halving HBM
    traffic vs f32.
  * qT, kT produced as [dout, tok] (transposed) by matmuls with the weight
    tiles as the stationary operand; v produced per-batch in natural
    [tok, dout] layout (needed as the moving operand of p @ v).
  * scores are computed transposed: S' = [k, q] = kT_h.T @ qT_h, which is
    exactly the layout the sf-net matmuls want (contraction over k).
  * sf-net + softmax are PHASE-BATCHED per group of 8 batches to amortize
    the ACT LUT table reloads (gelu and exp live in different ACT table
    sets; each switch costs ~1.3-2.7us):
      phase A (per batch): S' -> Ssb copy -> h1 -> gelu -> h2 accumulated
        back into the S' PSUM -> S2 copied to SBUF f32 (frees the PSUM).
      phase B: all 8 exps back-to-back (one exp-table load per group),
        interleaved with the 8 v-projections so the PE stays busy (and
        warm) while the scalar engine runs the exps.
      phase C (per batch): attention matmuls + softmax normalization +
        PE transpose of the attn rows into attnT [D, tok].
    This gives 2 table loads per group (16 total) instead of 2 per pair
    of batches (64 total).
  * attention: one matmul per head with stationary E'_h = exp(scores2)
    and moving [v_h | 1] producing [q, dh] plus the softmax denominator;
    normalization by per-partition (per-q) reciprocal on vector engine.
  * attn rows are transposed back per batch with the PE transpose and
    assembled into attnT [D, tok] feeding the output projection.
  * biases: bq/bk folded into the PSUM->SBUF copies (scalar engine bias),
    bv folded into the output bias (softmax rows sum to 1, so
    p @ (v + 1 bv^T) = p @ v + 1 bv^T), bo_eff = bo + bv @ Wo computed
    on device once.

All matmuls run in bf16 (fp32 PSUM accumulation); measured end-to-end
scale-relative error vs the fp32 reference ~4e-3.
"""

import contextlib
import os

import numpy as np
import ml_dtypes

import concourse.bass as bass
import concourse.mybir as mybir
import concourse.tile as tile
from concourse import bacc
from concourse import bass_utils

F32 = mybir.dt.float32
BF16 = mybir.dt.bfloat16
AF = mybir.ActivationFunctionType

N_CORES = 8
B, L, D, H = 512, 71, 512, 8
DH = D // H  # 64
FF = 128  # sf_net hidden
BC = B // N_CORES  # 64 batches per core
T = BC * L  # 4544 tokens per core
GB = 8  # batches per group
G = BC // GB  # 8 groups
GT = GB * L  # 568 tokens per group
HALF = GT // 2  # 284

_CACHE = {}


def _build():
    stage = int(os.environ.get("KSTAGE", "99"))
    nc = bacc.Bacc("TRN2", target_bir_lowering=False, debug=False,
                   num_devices=N_CORES)

    xqT = nc.dram_tensor("xqT", [D, T], BF16, kind="ExternalInput").ap()
    xkT = nc.dram_tensor("xkT", [D, T], BF16, kind="ExternalInput").ap()
    xvT = nc.dram_tensor("xvT", [D, T], BF16, kind="ExternalInput").ap()
    Wq = nc.dram_tensor("Wq", [D, D], BF16, kind="ExternalInput").ap()
    Wk = nc.dram_tensor("Wk", [D, D], BF16, kind="ExternalInput").ap()
    Wv = nc.dram_tensor("Wv", [D, D], BF16, kind="ExternalInput").ap()
    Wo = nc.dram_tensor("Wo", [D, D], BF16, kind="ExternalInput").ap()
    bq = nc.dram_tensor("bq", [D], F32, kind="ExternalInput").ap()
    bk = nc.dram_tensor("bk", [D], F32, kind="ExternalInput").ap()
    bv = nc.dram_tensor("bv", [D], F32, kind="ExternalInput").ap()
    bo = nc.dram_tensor("bo", [D], F32, kind="ExternalInput").ap()
    w1 = nc.dram_tensor("sf_w1", [L, FF], BF16, kind="ExternalInput").ap()
    b1 = nc.dram_tensor("sf_b1", [FF], F32, kind="ExternalInput").ap()
    w2 = nc.dram_tensor("sf_w2", [FF, L], BF16, kind="ExternalInput").ap()
    b2 = nc.dram_tensor("sf_b2", [L], F32, kind="ExternalInput").ap()
    out_d = nc.dram_tensor("out", [T, D], F32, kind="ExternalOutput").ap()

    with tile.TileContext(nc) as tc, contextlib.ExitStack() as ctx:
        singles = ctx.enter_context(tc.tile_pool(name="singles", bufs=1))
        p_xt = ctx.enter_context(tc.tile_pool(name="xt", bufs=2))
        p_qk = ctx.enter_context(tc.tile_pool(name="qk", bufs=2))
        p_v = ctx.enter_context(tc.tile_pool(name="v", bufs=GB))
        p_ssb = ctx.enter_context(tc.tile_pool(name="ssb", bufs=2))
        p_s2 = ctx.enter_context(tc.tile_pool(name="s2", bufs=GB + 1))
        p_esb = ctx.enter_context(tc.tile_pool(name="esb", bufs=GB))
        p_gel = ctx.enter_context(tc.tile_pool(name="gel", bufs=2))
        p_rec = ctx.enter_context(tc.tile_pool(name="rec", bufs=2))
        p_osb = ctx.enter_context(tc.tile_pool(name="osb", bufs=3))
        p_small = ctx.enter_context(tc.tile_pool(name="small", bufs=4))
        # PSUM: 8 banks total.
        #   sc: 2 bufs x 2 banks -- S' scores (phase A); in phase C the same
        #       ring hosts [attn pa | den] pairs (pa in bank 0, den in bank 1).
        #   h1: 2 bufs x 1 bank  -- sf-net hidden halves.
        #   pp: 2 bufs x 1 bank  -- v projections (B) + output projection.
        ps_pp = ctx.enter_context(tc.tile_pool(name="pp", bufs=2, space="PSUM"))
        ps_sc = ctx.enter_context(tc.tile_pool(name="sc", bufs=2, space="PSUM"))
        ps_h1 = ctx.enter_context(tc.tile_pool(name="h1", bufs=2, space="PSUM"))

        # ---- constants / weights ----
        def w_tiles(w_ap, name):
            t = singles.tile([128, 4, D], BF16, tag=f"w_{name}")
            nc.gpsimd.dma_start(out=t, in_=w_ap.rearrange("(j p) d -> p j d", p=128))
            return t

        Wq_sb = w_tiles(Wq, "q")
        Wk_sb = w_tiles(Wk, "k")
        Wv_sb = w_tiles(Wv, "v")
        Wo_sb = w_tiles(Wo, "o")

        def b_tile(b_ap, name):
            t = singles.tile([128, 4], F32, tag=f"b_{name}")
            nc.gpsimd.dma_start(out=t, in_=b_ap.rearrange("(j p) -> p j", p=128))
            return t

        bq_sb = b_tile(bq, "q")
        bk_sb = b_tile(bk, "k")
        bq8_sb = singles.tile([128, 4], F32, tag="bq8")
        nc.scalar.mul(bq8_sb, bq_sb, 0.125)

        w1_sb = singles.tile([L, FF], BF16, tag="w1")
        nc.gpsimd.dma_start(out=w1_sb, in_=w1)
        w2_sb = singles.tile([FF, L], BF16, tag="w2")
        nc.gpsimd.dma_start(out=w2_sb, in_=w2)
        b1_sb = singles.tile([FF, 1], F32, tag="b1")
        nc.gpsimd.dma_start(out=b1_sb, in_=b1.rearrange("(p o) -> p o", o=1))
        b2_sb = singles.tile([L, 1], F32, tag="b2")
        nc.gpsimd.dma_start(out=b2_sb, in_=b2.rearrange("(p o) -> p o", o=1))

        # all-ones stationary for the softmax-denominator matmuls: each
        # output row r of ones64.T @ E' is sum_k E'[k, :], i.e. the per-
        # column denominator replicated across 64 partitions.
        ones64 = singles.tile([L, DH], BF16, tag="ones64")
        nc.gpsimd.memset(ones64, 1.0)

        # bo_eff = bo + bv @ Wo, replicated to [128, D]
        bv_sb = singles.tile([128, 4], BF16, tag="bv")
        nc.gpsimd.dma_start(out=bv_sb, in_=bv.rearrange("(j p) -> p j", p=128))
        bo_sb = singles.tile([1, D], F32, tag="bo")
        nc.gpsimd.dma_start(out=bo_sb, in_=bo.rearrange("(o d) -> o d", o=1))
        ps_bvwo = ps_pp.tile([1, D], F32, tag="pp")
        for j in range(4):
            nc.tensor.matmul(ps_bvwo, bv_sb[:, j:j + 1], Wo_sb[:, j, :],
                             start=(j == 0), stop=(j == 3))
        boeff_row = singles.tile([1, D], F32, tag="boeffrow")
        nc.vector.tensor_add(boeff_row, ps_bvwo, bo_sb)
        ones_f32 = singles.tile([1, 128], F32, tag="ones1")
        nc.vector.memset(ones_f32, 1.0)
        ps_rep = ps_pp.tile([128, D], F32, tag="pp")
        nc.tensor.matmul(ps_rep, ones_f32, boeff_row, start=True, stop=True)
        BO_sb = singles.tile([128, D], F32, tag="BO")
        nc.vector.tensor_copy(out=BO_sb, in_=ps_rep)

        attnT = singles.tile([128, 4, T], BF16, tag="attnT")

        xq3 = xqT.rearrange("(j p) t -> p j t", p=128)
        xk3 = xkT.rearrange("(j p) t -> p j t", p=128)
        xv3 = xvT.rearrange("(j p) t -> p j t", p=128)

        # output projection out = attnT.T @ Wo + bo_eff, emitted in
        # 128-token chunks as soon as the covering groups are done (the
        # Tile dependency tracker gates each chunk on the attnT region it
        # reads), so the tail does not serialize after the groups.
        n_chunks = (T + 127) // 128 if stage >= 10 else 0
        chunks_done = 0

        def emit_out_chunks(upto_tokens):
            nonlocal chunks_done
            while (chunks_done < n_chunks
                   and (min(128 * (chunks_done + 1), T) <= upto_tokens)):
                c = chunks_done
                w = min(128, T - c * 128)
                po = ps_pp.tile([128, D], F32, tag="pp")
                for j in range(4):
                    nc.tensor.matmul(po[0:w],
                                     attnT[:, j, c * 128:c * 128 + w],
                                     Wo_sb[:, j, :],
                                     start=(j == 0), stop=(j == 3))
                osb = p_osb.tile([128, D], F32, tag="osb")
                nc.vector.tensor_add(osb[0:w], po[0:w], BO_sb[0:w])
                nc.sync.dma_start(out=out_d[c * 128:c * 128 + w, :],
                                  in_=osb[0:w])
                chunks_done += 1

        for g in range(G):
            if stage < 1:
                break
            t0 = g * GT
            xtq = p_xt.tile([128, 4, GT], BF16, tag="xtq")
            xtk = p_xt.tile([128, 4, GT], BF16, tag="xtk")
            xtv = p_xt.tile([128, 4, GT], BF16, tag="xtv")
            nc.gpsimd.dma_start(out=xtq, in_=xq3[:, :, t0:t0 + GT])
            nc.gpsimd.dma_start(out=xtk, in_=xk3[:, :, t0:t0 + GT])
            nc.gpsimd.dma_start(out=xtv, in_=xv3[:, :, t0:t0 + GT])

            # --- q/k projections (transposed layout [dout, tok]) ---
            qT = p_qk.tile([128, 4, GT], BF16, tag="qT")
            kT = p_qk.tile([128, 4, GT], BF16, tag="kT")
            for dt_ in range(4):
                for hf in range(2):
                    c0 = hf * HALF
                    pq = ps_pp.tile([128, HALF], F32, tag="pp")
                    for j in range(4):
                        nc.tensor.matmul(
                            pq, Wq_sb[:, j, dt_ * 128:(dt_ + 1) * 128],
                            xtq[:, j, c0:c0 + HALF],
                            start=(j == 0), stop=(j == 3))
                    nc.scalar.activation(
                        out=qT[:, dt_, c0:c0 + HALF], in_=pq, func=AF.Identity,
                        bias=bq8_sb[:, dt_:dt_ + 1], scale=0.125)
                    pk = ps_pp.tile([128, HALF], F32, tag="pp")
                    for j in range(4):
                        nc.tensor.matmul(
                            pk, Wk_sb[:, j, dt_ * 128:(dt_ + 1) * 128],
                            xtk[:, j, c0:c0 + HALF],
                            start=(j == 0), stop=(j == 3))
                    # bias-add on the scalar engine (Identity is in every
                    # ACT table set, so no table reload) to offload the DVE
                    nc.scalar.activation(
                        out=kT[:, dt_, c0:c0 + HALF], in_=pk, func=AF.Identity,
                        bias=bk_sb[:, dt_:dt_ + 1], scale=1.0)

            if stage < 3:
                continue

            # --- phase A: scores + sf-net hidden + residual, per batch ---
            # All gelus of the group run back-to-back on the scalar engine
            # (one gelu-table load per group); the post-residual scores S2
            # are copied to SBUF f32, freeing the S' PSUM banks.
            s2q = {}
            for bl in range(GB):
                tc0 = bl * L

                # --- scores S' = [k, q] ---
                # PE row groups must NOT alternate between matmuls (HW
                # wedge observed when the base partition flips 0<->64
                # inside a bank group), so heads are emitted parity-
                # grouped: bank 0 hosts even heads (qkT partition base
                # 0), bank 1 odd heads (base 64).  Head h lives at
                # column 512*(h%2) + 71*(h//2).
                S_ps = ps_sc.tile([L, 1024], F32, tag="sc")
                for i in range(2):
                    for hh in range(4):  # head 2*hh+i
                        off = 512 * i + L * hh
                        nc.tensor.matmul(
                            S_ps[:, off:off + L],
                            kT[64 * i:64 * i + 64, hh, tc0:tc0 + L],
                            qT[64 * i:64 * i + 64, hh, tc0:tc0 + L],
                            start=(hh == 0), stop=False)
                S3 = S_ps.rearrange("p (b r) -> p b r", b=2)[:, :, 0:4 * L]
                Ssb = p_ssb.tile([L, 2, 4 * L], BF16, tag="Ssb")
                nc.vector.tensor_copy(out=Ssb, in_=S3)
                Sflat = Ssb.rearrange("p b r -> p (b r)")

                # --- sf-net hidden layer + output, in half-batch (1-PSUM-
                # bank) pieces aligned to the score banks ---
                gel = p_gel.tile([FF, GT], BF16, tag="gel")
                for hf in range(2):
                    # padded to a full PSUM bank so the ring slots stay
                    # bank-aligned (matmul output cannot straddle banks)
                    h1_ps = ps_h1.tile([FF, 512], F32, tag="h1")
                    nc.tensor.matmul(h1_ps[:, 0:4 * L], w1_sb,
                                     Sflat[:, 4 * L * hf:4 * L * (hf + 1)],
                                     start=True, stop=True)
                    nc.scalar.activation(
                        out=gel[:, 4 * L * hf:4 * L * (hf + 1)],
                        in_=h1_ps[:, 0:4 * L],
                        func=AF.Gelu, bias=b1_sb, scale=1.0)
                    nc.tensor.matmul(
                        S_ps[:, 512 * hf:512 * hf + 4 * L], w2_sb,
                        gel[:, 4 * L * hf:4 * L * (hf + 1)],
                        start=False, stop=True)
                # S2 to SBUF (f32, exact) -- frees the S' PSUM pair and
                # decouples the exp phase from PSUM lifetimes.
                S2sb = p_s2.tile([L, 2, 4 * L], F32, tag="S2")
                nc.vector.tensor_copy(out=S2sb, in_=S3)
                s2q[bl] = S2sb
                last_gel = gel

            # The Tile scheduler orders each engine's queue by estimated
            # readiness, not emission order, so without a real dependency
            # it interleaves the exps between the gelus and walrus emits an
            # ACT table load per switch (~1.3us each).  Force every exp
            # after the group's last gelu by routing the exp bias through a
            # copy that reads (a scrap of) the last gel tile: b2g = 0*gel+b2.
            b2g = p_small.tile([L, 1], F32, tag="b2g")
            nc.vector.scalar_tensor_tensor(
                out=b2g, in0=last_gel[0:L, 0:1], scalar=0.0, in1=b2_sb,
                op0=mybir.AluOpType.mult, op1=mybir.AluOpType.add)

            # --- phase B: softmax exp (batched) + v projections ---
            # The 8 exps run back-to-back on the scalar engine (one
            # exp-table load per group); the v projections are independent
            # PE work interleaved so the PE stays warm during the exps.
            Ef = {}
            vq = {}
            for bl in range(GB):
                E_sb = p_esb.tile([L, 2, 4 * L], BF16, tag="E")
                nc.scalar.activation(out=E_sb, in_=s2q[bl], func=AF.Exp,
                                     bias=b2g, scale=1.0)
                Ef[bl] = E_sb.rearrange("p b r -> p (b r)")

                tc0 = bl * L
                pv = ps_pp.tile([L, D], F32, tag="pp")
                for j in range(4):
                    nc.tensor.matmul(pv, xtv[:, j, tc0:tc0 + L],
                                     Wv_sb[:, j, :],
                                     start=(j == 0), stop=(j == 3))
                v_sb = p_v.tile([L, H, DH], BF16, tag="v")
                nc.vector.tensor_copy(out=v_sb,
                                      in_=pv.rearrange("p (h d) -> p h d", h=H))
                vq[bl] = v_sb

            # --- phase C: attention, directly in attnT layout ---
            # attnT_h = v_h.T @ E'_h needs no transposes: stationary v_h
            # [k, dh], moving E'_h [k, q] -> out [dh, q].  Head pairs
            # (2*dt, 2*dt+1) land at partition rows 0-63 / 64-127 (PE
            # column tiling) of the dt-th 128-row attnT tile.  The
            # denominators ride as a second matmul with an all-ones
            # stationary: den[r, (hh q)] = sum_k E'[k, (hh q)] for every
            # r, i.e. the reciprocal multiplier pre-replicated across
            # partitions.  pa lives in bank 0 and den in bank 1 of one
            # 2-bank "sc"-ring allocation.
            for bl in range(GB):
                bi = g * GB + bl
                v_sb = vq.pop(bl)
                Eflat = Ef.pop(bl)
                sc_t = ps_sc.tile([128, 1024], F32, tag="sc")
                pa = sc_t[:, 0:4 * L].rearrange("p (d q) -> p d q", d=4)
                den = sc_t[:, 512:512 + 4 * L].rearrange(
                    "p (d q) -> p d q", d=4)
                for h in range(H):
                    rowb = 64 * (h % 2)
                    off_e = 4 * L * (h % 2) + L * (h // 2)
                    nc.tensor.matmul(
                        pa[rowb:rowb + DH, h // 2, :],
                        v_sb[:, h, :], Eflat[:, off_e:off_e + L],
                        start=True, stop=True, tile_position=(0, rowb))
                nc.tensor.matmul(den[0:64], ones64, Eflat[:, 0:4 * L],
                                 start=True, stop=True, tile_position=(0, 0))
                nc.tensor.matmul(den[64:128], ones64, Eflat[:, 4 * L:8 * L],
                                 start=True, stop=True, tile_position=(0, 64))
                recipd = p_rec.tile([128, 4, L], F32, tag="recipd")
                nc.vector.reciprocal_approx_fast(out=recipd, in_=den)
                nc.vector.tensor_mul(
                    attnT[:, :, bi * L:(bi + 1) * L], pa, recipd)
                emit_out_chunks((bi + 1) * L)

        emit_out_chunks(T)

    nc.compile()
    return nc


def _get_nc():
    if "nc" not in _CACHE:
        _CACHE["nc"] = _build()
    return _CACHE["nc"]


def _prep_in_maps(inputs):
    f32 = lambda a: np.ascontiguousarray(np.asarray(a, dtype=np.float32))
    bf16 = lambda a: np.ascontiguousarray(
        np.asarray(a, dtype=np.float32).astype(ml_dtypes.bfloat16))
    shared = {k: f32(inputs[k]) for k in ("bq", "bk", "bv", "bo",
                                          "sf_b1", "sf_b2")}
    shared.update({k: bf16(inputs[k]) for k in
                   ("Wq", "Wk", "Wv", "Wo", "sf_w1", "sf_w2")})
    xT = {}
    for key, name in (("query", "xqT"), ("key", "xkT"), ("value", "xvT")):
        # [B, L, D] -> [D, B, L], feature-major (layout change only)
        xT[name] = np.asarray(inputs[key], dtype=np.float32).astype(
            ml_dtypes.bfloat16).transpose(2, 0, 1)
    in_maps = []
    for c in range(N_CORES):
        m = dict(shared)
        for name in ("xqT", "xkT", "xvT"):
            m[name] = np.ascontiguousarray(
                xT[name][:, c * BC:(c + 1) * BC, :]).reshape(D, T)
        in_maps.append(m)
    return in_maps


def run(inputs, trace=False):
    nc = _get_nc()
    in_maps = _prep_in_maps(inputs)
    res = bass_utils.run_bass_kernel_spmd(
        nc, in_maps, core_ids=list(range(N_CORES)), trace=trace)
    out = np.concatenate(
        [res.results[c]["out"].reshape(BC, L, D) for c in range(N_CORES)],
        axis=0)
    return out, res


def kernel(**inputs) -> np.ndarray:
    out, _ = run(inputs, trace=False)
    return out


# revision 18
# speedup vs baseline: 1.0636x; 1.0636x over previous
"""Trainium2 Bass kernel for nn_MultiHeadedAttention_25984552141341.

Computation (reference):
    q/k/v = (x @ W + b) split into 8 heads of 64
    scores = q @ k^T / 8
    scores += sf_net(scores)   (SoftmaxResNet over the key dim, 71)
    p = softmax(scores, axis=key)
    out = (p @ v merged) @ Wo + bo

Sharding: batch (512) split across 8 NeuronCores, 64 batches each.
All weights replicated. Each core runs an identical Bass program (SPMD).

Device-side layout strategy (per core):
  * activations live feature-on-partition / token-on-free ("transposed"
    layout, xT = [D, B*L]); the host passes query/key/value pre-transposed
    AND pre-cast to bf16 (the matmuls run in bf16 anyway), halving HBM
    traffic vs f32.
  * qT, kT produced as [dout, tok] (transposed) by matmuls with the weight
    tiles as the stationary operand; v produced per-batch in natural
    [tok, dout] layout (needed as the moving operand of p @ v).
  * scores are computed transposed: S' = [k, q] = kT_h.T @ qT_h, which is
    exactly the layout the sf-net matmuls want (contraction over k).
  * sf-net + softmax are PHASE-BATCHED per group of 8 batches to amortize
    the ACT LUT table reloads (gelu and exp live in different ACT table
    sets; each switch costs ~1.3-2.7us):
      phase A (per batch): S' -> Ssb copy -> h1 -> gelu -> h2 accumulated
        back into the S' PSUM -> S2 copied to SBUF f32 (frees the PSUM).
      phase B: all 8 exps back-to-back (one exp-table load per group),
        interleaved with the 8 v-projections so the PE stays busy (and
        warm) while the scalar engine runs the exps.
      phase C (per batch): attention matmuls + softmax normalization +
        PE transpose of the attn rows into attnT [D, tok].
    This gives 2 table loads per group (16 total) instead of 2 per pair
    of batches (64 total).
  * attention: one matmul per head with stationary E'_h = exp(scores2)
    and moving [v_h | 1] producing [q, dh] plus the softmax denominator;
    normalization by per-partition (per-q) reciprocal on vector engine.
  * attn rows are transposed back per batch with the PE transpose and
    assembled into attnT [D, tok] feeding the output projection.
  * biases: bq/bk folded into the PSUM->SBUF copies (scalar engine bias),
    bv folded into the output bias (softmax rows sum to 1, so
    p @ (v + 1 bv^T) = p @ v + 1 bv^T), bo_eff = bo + bv @ Wo computed
    on device once.

All matmuls run in bf16 (fp32 PSUM accumulation); measured end-to-end
scale-relative error vs the fp32 reference ~4e-3.
"""

import contextlib
import os

import numpy as np
import ml_dtypes

import concourse.bass as bass
import concourse.mybir as mybir
import concourse.tile as tile
from concourse import bacc
from concourse import bass_utils

F32 = mybir.dt.float32
BF16 = mybir.dt.bfloat16
AF = mybir.ActivationFunctionType

N_CORES = 8
B, L, D, H = 512, 71, 512, 8
DH = D // H  # 64
FF = 128  # sf_net hidden
BC = B // N_CORES  # 64 batches per core
T = BC * L  # 4544 tokens per core
GB = 8  # batches per group
G = BC // GB  # 8 groups
GT = GB * L  # 568 tokens per group
HALF = GT // 2  # 284

_CACHE = {}


def _build():
    stage = int(os.environ.get("KSTAGE", "99"))
    nc = bacc.Bacc("TRN2", target_bir_lowering=False, debug=False,
                   num_devices=N_CORES)

    xqT = nc.dram_tensor("xqT", [D, T], BF16, kind="ExternalInput").ap()
    xkT = nc.dram_tensor("xkT", [D, T], BF16, kind="ExternalInput").ap()
    xvT = nc.dram_tensor("xvT", [D, T], BF16, kind="ExternalInput").ap()
    Wq = nc.dram_tensor("Wq", [D, D], BF16, kind="ExternalInput").ap()
    Wk = nc.dram_tensor("Wk", [D, D], BF16, kind="ExternalInput").ap()
    Wv = nc.dram_tensor("Wv", [D, D], BF16, kind="ExternalInput").ap()
    Wo = nc.dram_tensor("Wo", [D, D], BF16, kind="ExternalInput").ap()
    bq = nc.dram_tensor("bq", [D], F32, kind="ExternalInput").ap()
    bk = nc.dram_tensor("bk", [D], F32, kind="ExternalInput").ap()
    bv = nc.dram_tensor("bv", [D], F32, kind="ExternalInput").ap()
    bo = nc.dram_tensor("bo", [D], F32, kind="ExternalInput").ap()
    w1 = nc.dram_tensor("sf_w1", [L, FF], BF16, kind="ExternalInput").ap()
    b1 = nc.dram_tensor("sf_b1", [FF], F32, kind="ExternalInput").ap()
    w2 = nc.dram_tensor("sf_w2", [FF, L], BF16, kind="ExternalInput").ap()
    b2 = nc.dram_tensor("sf_b2", [L], F32, kind="ExternalInput").ap()
    out_d = nc.dram_tensor("out", [T, D], F32, kind="ExternalOutput").ap()

    with tile.TileContext(nc) as tc, contextlib.ExitStack() as ctx:
        singles = ctx.enter_context(tc.tile_pool(name="singles", bufs=1))
        p_xt = ctx.enter_context(tc.tile_pool(name="xt", bufs=2))
        p_qk = ctx.enter_context(tc.tile_pool(name="qk", bufs=2))
        p_v = ctx.enter_context(tc.tile_pool(name="v", bufs=GB))
        p_ssb = ctx.enter_context(tc.tile_pool(name="ssb", bufs=2))
        p_s2 = ctx.enter_context(tc.tile_pool(name="s2", bufs=GB + 1))
        p_esb = ctx.enter_context(tc.tile_pool(name="esb", bufs=GB))
        p_gel = ctx.enter_context(tc.tile_pool(name="gel", bufs=2))
        p_rec = ctx.enter_context(tc.tile_pool(name="rec", bufs=2))
        p_osb = ctx.enter_context(tc.tile_pool(name="osb", bufs=3))
        p_small = ctx.enter_context(tc.tile_pool(name="small", bufs=4))
        # PSUM: 8 banks total.
        #   sc: 2 bufs x 2 banks -- S' scores (phase A); in phase C the same
        #       ring hosts [attn pa | den] pairs (pa in bank 0, den in bank 1).
        #   h1: 2 bufs x 1 bank  -- sf-net hidden halves.
        #   pp: 2 bufs x 1 bank  -- v projections (B) + output projection.
        ps_pp = ctx.enter_context(tc.tile_pool(name="pp", bufs=2, space="PSUM"))
        ps_sc = ctx.enter_context(tc.tile_pool(name="sc", bufs=2, space="PSUM"))
        ps_h1 = ctx.enter_context(tc.tile_pool(name="h1", bufs=2, space="PSUM"))

        # ---- constants / weights ----
        def w_tiles(w_ap, name):
            t = singles.tile([128, 4, D], BF16, tag=f"w_{name}")
            nc.gpsimd.dma_start(out=t, in_=w_ap.rearrange("(j p) d -> p j d", p=128))
            return t

        Wq_sb = w_tiles(Wq, "q")
        Wk_sb = w_tiles(Wk, "k")
        Wv_sb = w_tiles(Wv, "v")
        Wo_sb = w_tiles(Wo, "o")

        def b_tile(b_ap, name):
            t = singles.tile([128, 4], F32, tag=f"b_{name}")
            nc.gpsimd.dma_start(out=t, in_=b_ap.rearrange("(j p) -> p j", p=128))
            return t

        bq_sb = b_tile(bq, "q")
        bk_sb = b_tile(bk, "k")
        bq8_sb = singles.tile([128, 4], F32, tag="bq8")
        nc.scalar.mul(bq8_sb, bq_sb, 0.125)

        w1_sb = singles.tile([L, FF], BF16, tag="w1")
        nc.gpsimd.dma_start(out=w1_sb, in_=w1)
        w2_sb = singles.tile([FF, L], BF16, tag="w2")
        nc.gpsimd.dma_start(out=w2_sb, in_=w2)
        b1_sb = singles.tile([FF, 1], F32, tag="b1")
        nc.gpsimd.dma_start(out=b1_sb, in_=b1.rearrange("(p o) -> p o", o=1))
        b2_sb = singles.tile([L, 1], F32, tag="b2")
        nc.gpsimd.dma_start(out=b2_sb, in_=b2.rearrange("(p o) -> p o", o=1))

        # all-ones stationary for the softmax-denominator matmuls: each
        # output row r of ones64.T @ E' is sum_k E'[k, :], i.e. the per-
        # column denominator replicated across 64 partitions.
        ones64 = singles.tile([L, DH], BF16, tag="ones64")
        nc.gpsimd.memset(ones64, 1.0)

        # bo_eff = bo + bv @ Wo, replicated to [128, D]
        bv_sb = singles.tile([128, 4], BF16, tag="bv")
        nc.gpsimd.dma_start(out=bv_sb, in_=bv.rearrange("(j p) -> p j", p=128))
        bo_sb = singles.tile([1, D], F32, tag="bo")
        nc.gpsimd.dma_start(out=bo_sb, in_=bo.rearrange("(o d) -> o d", o=1))
        ps_bvwo = ps_pp.tile([1, D], F32, tag="pp")
        for j in range(4):
            nc.tensor.matmul(ps_bvwo, bv_sb[:, j:j + 1], Wo_sb[:, j, :],
                             start=(j == 0), stop=(j == 3))
        boeff_row = singles.tile([1, D], F32, tag="boeffrow")
        nc.vector.tensor_add(boeff_row, ps_bvwo, bo_sb)
        ones_f32 = singles.tile([1, 128], F32, tag="ones1")
        nc.vector.memset(ones_f32, 1.0)
        ps_rep = ps_pp.tile([128, D], F32, tag="pp")
        nc.tensor.matmul(ps_rep, ones_f32, boeff_row, start=True, stop=True)
        BO_sb = singles.tile([128, D], F32, tag="BO")
        nc.vector.tensor_copy(out=BO_sb, in_=ps_rep)

        attnT = singles.tile([128, 4, T], BF16, tag="attnT")

        xq3 = xqT.rearrange("(j p) t -> p j t", p=128)
        xk3 = xkT.rearrange("(j p) t -> p j t", p=128)
        xv3 = xvT.rearrange("(j p) t -> p j t", p=128)

        for g in range(G):
            if stage < 1:
                break
            t0 = g * GT
            xtq = p_xt.tile([128, 4, GT], BF16, tag="xtq")
            xtk = p_xt.tile([128, 4, GT], BF16, tag="xtk")
            xtv = p_xt.tile([128, 4, GT], BF16, tag="xtv")
            nc.gpsimd.dma_start(out=xtq, in_=xq3[:, :, t0:t0 + GT])
            nc.gpsimd.dma_start(out=xtk, in_=xk3[:, :, t0:t0 + GT])
            nc.gpsimd.dma_start(out=xtv, in_=xv3[:, :, t0:t0 + GT])

            # --- q/k projections (transposed layout [dout, tok]) ---
            qT = p_qk.tile([128, 4, GT], BF16, tag="qT")
            kT = p_qk.tile([128, 4, GT], BF16, tag="kT")
            for dt_ in range(4):
                for hf in range(2):
                    c0 = hf * HALF
                    pq = ps_pp.tile([128, HALF], F32, tag="pp")
                    for j in range(4):
                        nc.tensor.matmul(
                            pq, Wq_sb[:, j, dt_ * 128:(dt_ + 1) * 128],
                            xtq[:, j, c0:c0 + HALF],
                            start=(j == 0), stop=(j == 3))
                    nc.scalar.activation(
                        out=qT[:, dt_, c0:c0 + HALF], in_=pq, func=AF.Identity,
                        bias=bq8_sb[:, dt_:dt_ + 1], scale=0.125)
                    pk = ps_pp.tile([128, HALF], F32, tag="pp")
                    for j in range(4):
                        nc.tensor.matmul(
                            pk, Wk_sb[:, j, dt_ * 128:(dt_ + 1) * 128],
                            xtk[:, j, c0:c0 + HALF],
                            start=(j == 0), stop=(j == 3))
                    # bias-add on the scalar engine (Identity is in every
                    # ACT table set, so no table reload) to offload the DVE
                    nc.scalar.activation(
                        out=kT[:, dt_, c0:c0 + HALF], in_=pk, func=AF.Identity,
                        bias=bk_sb[:, dt_:dt_ + 1], scale=1.0)

            if stage < 3:
                continue

            # --- phase A: scores + sf-net hidden + residual, per batch ---
            # All gelus of the group run back-to-back on the scalar engine
            # (one gelu-table load per group); the post-residual scores S2
            # are copied to SBUF f32, freeing the S' PSUM banks.
            s2q = {}
            for bl in range(GB):
                tc0 = bl * L

                # --- scores S' = [k, q] ---
                # PE row groups must NOT alternate between matmuls (HW
                # wedge observed when the base partition flips 0<->64
                # inside a bank group), so heads are emitted parity-
                # grouped: bank 0 hosts even heads (qkT partition base
                # 0), bank 1 odd heads (base 64).  Head h lives at
                # column 512*(h%2) + 71*(h//2).
                S_ps = ps_sc.tile([L, 1024], F32, tag="sc")
                for i in range(2):
                    for hh in range(4):  # head 2*hh+i
                        off = 512 * i + L * hh
                        nc.tensor.matmul(
                            S_ps[:, off:off + L],
                            kT[64 * i:64 * i + 64, hh, tc0:tc0 + L],
                            qT[64 * i:64 * i + 64, hh, tc0:tc0 + L],
                            start=(hh == 0), stop=False)
                S3 = S_ps.rearrange("p (b r) -> p b r", b=2)[:, :, 0:4 * L]
                Ssb = p_ssb.tile([L, 2, 4 * L], BF16, tag="Ssb")
                nc.vector.tensor_copy(out=Ssb, in_=S3)
                Sflat = Ssb.rearrange("p b r -> p (b r)")

                # --- sf-net hidden layer + output, in half-batch (1-PSUM-
                # bank) pieces aligned to the score banks ---
                gel = p_gel.tile([FF, GT], BF16, tag="gel")
                for hf in range(2):
                    # padded to a full PSUM bank so the ring slots stay
                    # bank-aligned (matmul output cannot straddle banks)
                    h1_ps = ps_h1.tile([FF, 512], F32, tag="h1")
                    nc.tensor.matmul(h1_ps[:, 0:4 * L], w1_sb,
                                     Sflat[:, 4 * L * hf:4 * L * (hf + 1)],
                                     start=True, stop=True)
                    nc.scalar.activation(
                        out=gel[:, 4 * L * hf:4 * L * (hf + 1)],
                        in_=h1_ps[:, 0:4 * L],
                        func=AF.Gelu, bias=b1_sb, scale=1.0)
                    nc.tensor.matmul(
                        S_ps[:, 512 * hf:512 * hf + 4 * L], w2_sb,
                        gel[:, 4 * L * hf:4 * L * (hf + 1)],
                        start=False, stop=True)
                # S2 to SBUF (f32, exact) -- frees the S' PSUM pair and
                # decouples the exp phase from PSUM lifetimes.
                S2sb = p_s2.tile([L, 2, 4 * L], F32, tag="S2")
                nc.vector.tensor_copy(out=S2sb, in_=S3)
                s2q[bl] = S2sb
                last_gel = gel

            # The Tile scheduler orders each engine's queue by estimated
            # readiness, not emission order, so without a real dependency
            # it interleaves the exps between the gelus and walrus emits an
            # ACT table load per switch (~1.3us each).  Force every exp
            # after the group's last gelu by routing the exp bias through a
            # copy that reads (a scrap of) the last gel tile: b2g = 0*gel+b2.
            b2g = p_small.tile([L, 1], F32, tag="b2g")
            nc.vector.scalar_tensor_tensor(
                out=b2g, in0=last_gel[0:L, 0:1], scalar=0.0, in1=b2_sb,
                op0=mybir.AluOpType.mult, op1=mybir.AluOpType.add)

            # --- phase B: softmax exp (batched) + v projections ---
            # The 8 exps run back-to-back on the scalar engine (one
            # exp-table load per group); the v projections are independent
            # PE work interleaved so the PE stays warm during the exps.
            Ef = {}
            vq = {}
            for bl in range(GB):
                E_sb = p_esb.tile([L, 2, 4 * L], BF16, tag="E")
                nc.scalar.activation(out=E_sb, in_=s2q[bl], func=AF.Exp,
                                     bias=b2g, scale=1.0)
                Ef[bl] = E_sb.rearrange("p b r -> p (b r)")

                tc0 = bl * L
                pv = ps_pp.tile([L, D], F32, tag="pp")
                for j in range(4):
                    nc.tensor.matmul(pv, xtv[:, j, tc0:tc0 + L],
                                     Wv_sb[:, j, :],
                                     start=(j == 0), stop=(j == 3))
                v_sb = p_v.tile([L, H, DH], BF16, tag="v")
                nc.vector.tensor_copy(out=v_sb,
                                      in_=pv.rearrange("p (h d) -> p h d", h=H))
                vq[bl] = v_sb

            # --- phase C: attention, directly in attnT layout ---
            # attnT_h = v_h.T @ E'_h needs no transposes: stationary v_h
            # [k, dh], moving E'_h [k, q] -> out [dh, q].  Head pairs
            # (2*dt, 2*dt+1) land at partition rows 0-63 / 64-127 (PE
            # column tiling) of the dt-th 128-row attnT tile.  The
            # denominators ride as a second matmul with an all-ones
            # stationary: den[r, (hh q)] = sum_k E'[k, (hh q)] for every
            # r, i.e. the reciprocal multiplier pre-replicated across
            # partitions.  pa borrows the h1 ring and den the pp ring
            # (both idle during phase C) so the sc ring stays free and
            # the next group's scores can overlap this phase.
            for bl in range(GB):
                bi = g * GB + bl
                v_sb = vq.pop(bl)
                Eflat = Ef.pop(bl)
                pa = ps_h1.tile([128, 4, L], F32, tag="h1")
                den = ps_pp.tile([128, 4, L], F32, tag="pp")
                for h in range(H):
                    rowb = 64 * (h % 2)
                    off_e = 4 * L * (h % 2) + L * (h // 2)
                    nc.tensor.matmul(
                        pa[rowb:rowb + DH, h // 2, :],
                        v_sb[:, h, :], Eflat[:, off_e:off_e + L],
                        start=True, stop=True, tile_position=(0, rowb))
                nc.tensor.matmul(den[0:64], ones64, Eflat[:, 0:4 * L],
                                 start=True, stop=True, tile_position=(0, 0))
                nc.tensor.matmul(den[64:128], ones64, Eflat[:, 4 * L:8 * L],
                                 start=True, stop=True, tile_position=(0, 64))
                recipd = p_rec.tile([128, 4, L], F32, tag="recipd")
                nc.vector.reciprocal_approx_fast(out=recipd, in_=den)
                nc.vector.tensor_mul(
                    attnT[:, :, bi * L:(bi + 1) * L], pa, recipd)

        # --- output projection out = attnT.T @ Wo + bo_eff ---
        n_chunks = (T + 127) // 128 if stage >= 10 else 0
        for c in range(n_chunks):
            w = min(128, T - c * 128)
            po = ps_pp.tile([128, D], F32, tag="pp")
            for j in range(4):
                nc.tensor.matmul(po[0:w], attnT[:, j, c * 128:c * 128 + w],
                                 Wo_sb[:, j, :], start=(j == 0), stop=(j == 3))
            osb = p_osb.tile([128, D], F32, tag="osb")
            nc.vector.tensor_add(osb[0:w], po[0:w], BO_sb[0:w])
            nc.sync.dma_start(out=out_d[c * 128:c * 128 + w, :], in_=osb[0:w])

    nc.compile()
    return nc


def _get_nc():
    if "nc" not in _CACHE:
        _CACHE["nc"] = _build()
    return _CACHE["nc"]


def _prep_in_maps(inputs):
    f32 = lambda a: np.ascontiguousarray(np.asarray(a, dtype=np.float32))
    bf16 = lambda a: np.ascontiguousarray(
        np.asarray(a, dtype=np.float32).astype(ml_dtypes.bfloat16))
    shared = {k: f32(inputs[k]) for k in ("bq", "bk", "bv", "bo",
                                          "sf_b1", "sf_b2")}
    shared.update({k: bf16(inputs[k]) for k in
                   ("Wq", "Wk", "Wv", "Wo", "sf_w1", "sf_w2")})
    xT = {}
    for key, name in (("query", "xqT"), ("key", "xkT"), ("value", "xvT")):
        # [B, L, D] -> [D, B, L], feature-major (layout change only)
        xT[name] = np.asarray(inputs[key], dtype=np.float32).astype(
            ml_dtypes.bfloat16).transpose(2, 0, 1)
    in_maps = []
    for c in range(N_CORES):
        m = dict(shared)
        for name in ("xqT", "xkT", "xvT"):
            m[name] = np.ascontiguousarray(
                xT[name][:, c * BC:(c + 1) * BC, :]).reshape(D, T)
        in_maps.append(m)
    return in_maps


def run(inputs, trace=False):
    nc = _get_nc()
    in_maps = _prep_in_maps(inputs)
    res = bass_utils.run_bass_kernel_spmd(
        nc, in_maps, core_ids=list(range(N_CORES)), trace=trace)
    out = np.concatenate(
        [res.results[c]["out"].reshape(BC, L, D) for c in range(N_CORES)],
        axis=0)
    return out, res


def kernel(**inputs) -> np.ndarray:
    out, _ = run(inputs, trace=False)
    return out


# revision 22
# speedup vs baseline: 1.1341x; 1.0663x over previous
"""Trainium2 Bass kernel for nn_MultiHeadedAttention_25984552141341.

Computation (reference):
    q/k/v = (x @ W + b) split into 8 heads of 64
    scores = q @ k^T / 8
    scores += sf_net(scores)   (SoftmaxResNet over the key dim, 71)
    p = softmax(scores, axis=key)
    out = (p @ v merged) @ Wo + bo

Sharding: batch (512) split across 8 NeuronCores, 64 batches each.
All weights replicated. Each core runs an identical Bass program (SPMD).

Device-side layout strategy (per core):
  * activations live feature-on-partition / token-on-free ("transposed"
    layout, xT = [D, B*L]); the host passes query/key/value pre-transposed
    AND pre-cast to bf16 (the matmuls run in bf16 anyway), halving HBM
    traffic vs f32.
  * qT, kT produced as [dout, tok] (transposed) by matmuls with the weight
    tiles as the stationary operand; v produced per-batch in natural
    [tok, dout] layout (needed as the moving operand of p @ v).
  * scores are computed transposed: S' = [k, q] = kT_h.T @ qT_h, which is
    exactly the layout the sf-net matmuls want (contraction over k).
  * sf-net + softmax are PHASE-BATCHED per group of 8 batches to amortize
    the ACT LUT table reloads (gelu and exp live in different ACT table
    sets; each switch costs ~1.3-2.7us):
      phase A (per batch): S' -> Ssb copy -> h1 -> gelu -> h2 accumulated
        back into the S' PSUM -> S2 copied to SBUF f32 (frees the PSUM).
      phase B: all 8 exps back-to-back (one exp-table load per group),
        interleaved with the 8 v-projections so the PE stays busy (and
        warm) while the scalar engine runs the exps.
      phase C (per batch): attention matmuls + softmax normalization +
        PE transpose of the attn rows into attnT [D, tok].
    This gives 2 table loads per group (16 total) instead of 2 per pair
    of batches (64 total).
  * attention: one matmul per head with stationary E'_h = exp(scores2)
    and moving [v_h | 1] producing [q, dh] plus the softmax denominator;
    normalization by per-partition (per-q) reciprocal on vector engine.
  * attn rows are transposed back per batch with the PE transpose and
    assembled into attnT [D, tok] feeding the output projection.
  * biases: bq/bk folded into the PSUM->SBUF copies (scalar engine bias),
    bv folded into the output bias (softmax rows sum to 1, so
    p @ (v + 1 bv^T) = p @ v + 1 bv^T), bo_eff = bo + bv @ Wo computed
    on device once.

All matmuls run in bf16 (fp32 PSUM accumulation); measured end-to-end
scale-relative error vs the fp32 reference ~4e-3.
"""

import contextlib
import os

import numpy as np
import ml_dtypes

import concourse.bass as bass
import concourse.mybir as mybir
import concourse.tile as tile
from concourse import bacc
from concourse import bass_utils

F32 = mybir.dt.float32
BF16 = mybir.dt.bfloat16
AF = mybir.ActivationFunctionType

N_CORES = 8
B, L, D, H = 512, 71, 512, 8
DH = D // H  # 64
FF = 128  # sf_net hidden
BC = B // N_CORES  # 64 batches per core
T = BC * L  # 4544 tokens per core
GB = 8  # batches per group
G = BC // GB  # 8 groups
GT = GB * L  # 568 tokens per group
HALF = GT // 2  # 284

_CACHE = {}


def _build():
    stage = int(os.environ.get("KSTAGE", "99"))
    nc = bacc.Bacc("TRN2", target_bir_lowering=False, debug=False,
                   num_devices=N_CORES)

    xqT = nc.dram_tensor("xqT", [D, T], BF16, kind="ExternalInput").ap()
    xkT = nc.dram_tensor("xkT", [D, T], BF16, kind="ExternalInput").ap()
    xvT = nc.dram_tensor("xvT", [D, T], BF16, kind="ExternalInput").ap()
    Wq = nc.dram_tensor("Wq", [D, D], BF16, kind="ExternalInput").ap()
    Wk = nc.dram_tensor("Wk", [D, D], BF16, kind="ExternalInput").ap()
    Wv = nc.dram_tensor("Wv", [D, D], BF16, kind="ExternalInput").ap()
    Wo = nc.dram_tensor("Wo", [D, D], BF16, kind="ExternalInput").ap()
    bq = nc.dram_tensor("bq", [D], F32, kind="ExternalInput").ap()
    bk = nc.dram_tensor("bk", [D], F32, kind="ExternalInput").ap()
    bv = nc.dram_tensor("bv", [D], F32, kind="ExternalInput").ap()
    bo = nc.dram_tensor("bo", [D], F32, kind="ExternalInput").ap()
    w1 = nc.dram_tensor("sf_w1", [L, FF], BF16, kind="ExternalInput").ap()
    b1 = nc.dram_tensor("sf_b1", [FF], F32, kind="ExternalInput").ap()
    w2 = nc.dram_tensor("sf_w2", [FF, L], BF16, kind="ExternalInput").ap()
    b2 = nc.dram_tensor("sf_b2", [L], F32, kind="ExternalInput").ap()
    out_d = nc.dram_tensor("out", [T, D], F32, kind="ExternalOutput").ap()

    with tile.TileContext(nc) as tc, contextlib.ExitStack() as ctx:
        singles = ctx.enter_context(tc.tile_pool(name="singles", bufs=1))
        p_xt = ctx.enter_context(tc.tile_pool(name="xt", bufs=3))
        p_qk = ctx.enter_context(tc.tile_pool(name="qk", bufs=2))
        p_v = ctx.enter_context(tc.tile_pool(name="v", bufs=GB))
        p_ssb = ctx.enter_context(tc.tile_pool(name="ssb", bufs=2))
        p_s2 = ctx.enter_context(tc.tile_pool(name="s2", bufs=GB + 1))
        p_esb = ctx.enter_context(tc.tile_pool(name="esb", bufs=GB))
        p_gel = ctx.enter_context(tc.tile_pool(name="gel", bufs=2))
        p_rec = ctx.enter_context(tc.tile_pool(name="rec", bufs=2))
        p_osb = ctx.enter_context(tc.tile_pool(name="osb", bufs=3))
        p_small = ctx.enter_context(tc.tile_pool(name="small", bufs=4))
        # PSUM: 8 banks total.
        #   sc: 2 bufs x 2 banks -- S' scores (phase A); in phase C the same
        #       ring hosts [attn pa | den] pairs (pa in bank 0, den in bank 1).
        #   h1: 2 bufs x 1 bank  -- sf-net hidden halves.
        #   pp: 2 bufs x 1 bank  -- v projections (B) + output projection.
        ps_pp = ctx.enter_context(tc.tile_pool(name="pp", bufs=2, space="PSUM"))
        ps_sc = ctx.enter_context(tc.tile_pool(name="sc", bufs=2, space="PSUM"))
        ps_h1 = ctx.enter_context(tc.tile_pool(name="h1", bufs=2, space="PSUM"))

        # ---- constants / weights ----
        def w_tiles(w_ap, name):
            t = singles.tile([128, 4, D], BF16, tag=f"w_{name}")
            nc.gpsimd.dma_start(out=t, in_=w_ap.rearrange("(j p) d -> p j d", p=128))
            return t

        Wq_sb = w_tiles(Wq, "q")
        Wk_sb = w_tiles(Wk, "k")
        Wv_sb = w_tiles(Wv, "v")
        Wo_sb = w_tiles(Wo, "o")

        def b_tile(b_ap, name):
            t = singles.tile([128, 4], F32, tag=f"b_{name}")
            nc.gpsimd.dma_start(out=t, in_=b_ap.rearrange("(j p) -> p j", p=128))
            return t

        bq_sb = b_tile(bq, "q")
        bk_sb = b_tile(bk, "k")
        bq8_sb = singles.tile([128, 4], F32, tag="bq8")
        nc.scalar.mul(bq8_sb, bq_sb, 0.125)

        w1_sb = singles.tile([L, FF], BF16, tag="w1")
        nc.gpsimd.dma_start(out=w1_sb, in_=w1)
        w2_sb = singles.tile([FF, L], BF16, tag="w2")
        nc.gpsimd.dma_start(out=w2_sb, in_=w2)
        b1_sb = singles.tile([FF, 1], F32, tag="b1")
        nc.gpsimd.dma_start(out=b1_sb, in_=b1.rearrange("(p o) -> p o", o=1))
        b2_sb = singles.tile([L, 1], F32, tag="b2")
        nc.gpsimd.dma_start(out=b2_sb, in_=b2.rearrange("(p o) -> p o", o=1))

        # all-ones stationary for the softmax-denominator matmuls: each
        # output row r of ones64.T @ E' is sum_k E'[k, :], i.e. the per-
        # column denominator replicated across 64 partitions.
        ones64 = singles.tile([L, DH], BF16, tag="ones64")
        nc.gpsimd.memset(ones64, 1.0)

        # bo_eff = bo + bv @ Wo, replicated to [128, D]
        bv_sb = singles.tile([128, 4], BF16, tag="bv")
        nc.gpsimd.dma_start(out=bv_sb, in_=bv.rearrange("(j p) -> p j", p=128))
        bo_sb = singles.tile([1, D], F32, tag="bo")
        nc.gpsimd.dma_start(out=bo_sb, in_=bo.rearrange("(o d) -> o d", o=1))
        ps_bvwo = ps_pp.tile([1, D], F32, tag="pp")
        for j in range(4):
            nc.tensor.matmul(ps_bvwo, bv_sb[:, j:j + 1], Wo_sb[:, j, :],
                             start=(j == 0), stop=(j == 3))
        boeff_row = singles.tile([1, D], F32, tag="boeffrow")
        nc.vector.tensor_add(boeff_row, ps_bvwo, bo_sb)
        ones_f32 = singles.tile([1, 128], F32, tag="ones1")
        nc.vector.memset(ones_f32, 1.0)
        ps_rep = ps_pp.tile([128, D], F32, tag="pp")
        nc.tensor.matmul(ps_rep, ones_f32, boeff_row, start=True, stop=True)
        BO_sb = singles.tile([128, D], F32, tag="BO")
        nc.vector.tensor_copy(out=BO_sb, in_=ps_rep)

        attnT = singles.tile([128, 4, T], BF16, tag="attnT")

        xq3 = xqT.rearrange("(j p) t -> p j t", p=128)
        xk3 = xkT.rearrange("(j p) t -> p j t", p=128)
        xv3 = xvT.rearrange("(j p) t -> p j t", p=128)

        for g in range(G):
            if stage < 1:
                break
            t0 = g * GT
            xtq = p_xt.tile([128, 4, GT], BF16, tag="xtq")
            xtk = p_xt.tile([128, 4, GT], BF16, tag="xtk")
            xtv = p_xt.tile([128, 4, GT], BF16, tag="xtv")
            nc.gpsimd.dma_start(out=xtq, in_=xq3[:, :, t0:t0 + GT])
            nc.gpsimd.dma_start(out=xtk, in_=xk3[:, :, t0:t0 + GT])
            nc.gpsimd.dma_start(out=xtv, in_=xv3[:, :, t0:t0 + GT])

            # --- q/k projections (transposed layout [dout, tok]) ---
            qT = p_qk.tile([128, 4, GT], BF16, tag="qT")
            kT = p_qk.tile([128, 4, GT], BF16, tag="kT")
            for dt_ in range(4):
                for hf in range(2):
                    c0 = hf * HALF
                    pq = ps_pp.tile([128, HALF], F32, tag="pp")
                    for j in range(4):
                        nc.tensor.matmul(
                            pq, Wq_sb[:, j, dt_ * 128:(dt_ + 1) * 128],
                            xtq[:, j, c0:c0 + HALF],
                            start=(j == 0), stop=(j == 3))
                    nc.scalar.activation(
                        out=qT[:, dt_, c0:c0 + HALF], in_=pq, func=AF.Identity,
                        bias=bq8_sb[:, dt_:dt_ + 1], scale=0.125)
                    pk = ps_pp.tile([128, HALF], F32, tag="pp")
                    for j in range(4):
                        nc.tensor.matmul(
                            pk, Wk_sb[:, j, dt_ * 128:(dt_ + 1) * 128],
                            xtk[:, j, c0:c0 + HALF],
                            start=(j == 0), stop=(j == 3))
                    # bias-add on the scalar engine (Identity is in every
                    # ACT table set, so no table reload) to offload the DVE
                    nc.scalar.activation(
                        out=kT[:, dt_, c0:c0 + HALF], in_=pk, func=AF.Identity,
                        bias=bk_sb[:, dt_:dt_ + 1], scale=1.0)

            if stage < 3:
                continue

            # --- phase A: scores + sf-net hidden + residual, per batch ---
            # All gelus of the group run back-to-back on the scalar engine
            # (one gelu-table load per group); the post-residual scores S2
            # are copied to SBUF f32, freeing the S' PSUM banks.
            s2q = {}
            for bl in range(GB):
                tc0 = bl * L

                # --- scores S' = [k, q] ---
                # PE row groups must NOT alternate between matmuls (HW
                # wedge observed when the base partition flips 0<->64
                # inside a bank group), so heads are emitted parity-
                # grouped: bank 0 hosts even heads (qkT partition base
                # 0), bank 1 odd heads (base 64).  Head h lives at
                # column 512*(h%2) + 71*(h//2).
                S_ps = ps_sc.tile([L, 1024], F32, tag="sc")
                for i in range(2):
                    for hh in range(4):  # head 2*hh+i
                        off = 512 * i + L * hh
                        nc.tensor.matmul(
                            S_ps[:, off:off + L],
                            kT[64 * i:64 * i + 64, hh, tc0:tc0 + L],
                            qT[64 * i:64 * i + 64, hh, tc0:tc0 + L],
                            start=(hh == 0), stop=False)
                S3 = S_ps.rearrange("p (b r) -> p b r", b=2)[:, :, 0:4 * L]
                Ssb = p_ssb.tile([L, 2, 4 * L], BF16, tag="Ssb")
                nc.vector.tensor_copy(out=Ssb, in_=S3)
                Sflat = Ssb.rearrange("p b r -> p (b r)")

                # --- sf-net hidden layer + output, in half-batch (1-PSUM-
                # bank) pieces aligned to the score banks ---
                gel = p_gel.tile([FF, GT], BF16, tag="gel")
                for hf in range(2):
                    # padded to a full PSUM bank so the ring slots stay
                    # bank-aligned (matmul output cannot straddle banks)
                    h1_ps = ps_h1.tile([FF, 512], F32, tag="h1")
                    nc.tensor.matmul(h1_ps[:, 0:4 * L], w1_sb,
                                     Sflat[:, 4 * L * hf:4 * L * (hf + 1)],
                                     start=True, stop=True)
                    nc.scalar.activation(
                        out=gel[:, 4 * L * hf:4 * L * (hf + 1)],
                        in_=h1_ps[:, 0:4 * L],
                        func=AF.Gelu, bias=b1_sb, scale=1.0)
                    nc.tensor.matmul(
                        S_ps[:, 512 * hf:512 * hf + 4 * L], w2_sb,
                        gel[:, 4 * L * hf:4 * L * (hf + 1)],
                        start=False, stop=True)
                # S2 to SBUF (f32, exact) -- frees the S' PSUM pair and
                # decouples the exp phase from PSUM lifetimes.
                S2sb = p_s2.tile([L, 2, 4 * L], F32, tag="S2")
                nc.vector.tensor_copy(out=S2sb, in_=S3)
                s2q[bl] = S2sb
                last_gel = gel

            # The Tile scheduler orders each engine's queue by estimated
            # readiness, not emission order, so without a real dependency
            # it interleaves the exps between the gelus and walrus emits an
            # ACT table load per switch (~1.3us each).  Force every exp
            # after the group's last gelu by routing the exp bias through a
            # copy that reads (a scrap of) the last gel tile: b2g = 0*gel+b2.
            b2g = p_small.tile([L, 1], F32, tag="b2g")
            nc.vector.scalar_tensor_tensor(
                out=b2g, in0=last_gel[0:L, 0:1], scalar=0.0, in1=b2_sb,
                op0=mybir.AluOpType.mult, op1=mybir.AluOpType.add)

            # --- phase BC: softmax exp + v projections + attention ---
            # The 8 exps run back-to-back on the scalar engine (one
            # exp-table load per group).  The v projections AND the
            # attention of the previous batch run on the PE in the same
            # window: together they keep the PE dense enough that the HAM
            # clock gate never re-throttles it to half rate (a ~10us K=4/8
            # window per group was measured with exp/attn as separate
            # phases).
            #
            # Attention is computed directly in attnT layout:
            # attnT_h = v_h.T @ E'_h needs no transposes: stationary v_h
            # [k, dh], moving E'_h [k, q] -> out [dh, q].  Head pairs
            # (2*dt, 2*dt+1) land at partition rows 0-63 / 64-127 (PE
            # column tiling) of the dt-th 128-row attnT tile.  The
            # denominators ride as a second matmul with an all-ones
            # stationary: den[r, (hh q)] = sum_k E'[k, (hh q)] for every
            # r, i.e. the reciprocal multiplier pre-replicated across
            # partitions.  pa lives in bank 0 and den in bank 1 of one
            # 2-bank "sc"-ring allocation.  The den matmuls go FIRST so
            # the DVE reciprocal runs concurrently with the attn matmuls.
            Ef = {}
            vq = {}

            def emit_attn(bl):
                bi = g * GB + bl
                v_sb = vq.pop(bl)
                Eflat = Ef.pop(bl)
                sc_t = ps_sc.tile([128, 1024], F32, tag="sc")
                pa = sc_t[:, 0:4 * L].rearrange("p (d q) -> p d q", d=4)
                den = sc_t[:, 512:512 + 4 * L].rearrange(
                    "p (d q) -> p d q", d=4)
                nc.tensor.matmul(den[0:64], ones64, Eflat[:, 0:4 * L],
                                 start=True, stop=True, tile_position=(0, 0))
                nc.tensor.matmul(den[64:128], ones64, Eflat[:, 4 * L:8 * L],
                                 start=True, stop=True, tile_position=(0, 64))
                for h in range(H):
                    rowb = 64 * (h % 2)
                    off_e = 4 * L * (h % 2) + L * (h // 2)
                    nc.tensor.matmul(
                        pa[rowb:rowb + DH, h // 2, :],
                        v_sb[:, h, :], Eflat[:, off_e:off_e + L],
                        start=True, stop=True, tile_position=(0, rowb))
                recipd = p_rec.tile([128, 4, L], F32, tag="recipd")
                nc.vector.reciprocal_approx_fast(out=recipd, in_=den)
                nc.vector.tensor_mul(
                    attnT[:, :, bi * L:(bi + 1) * L], pa, recipd)

            for bl in range(GB):
                E_sb = p_esb.tile([L, 2, 4 * L], BF16, tag="E")
                nc.scalar.activation(out=E_sb, in_=s2q[bl], func=AF.Exp,
                                     bias=b2g, scale=1.0)
                Ef[bl] = E_sb.rearrange("p b r -> p (b r)")

                tc0 = bl * L
                pv = ps_pp.tile([L, D], F32, tag="pp")
                for j in range(4):
                    nc.tensor.matmul(pv, xtv[:, j, tc0:tc0 + L],
                                     Wv_sb[:, j, :],
                                     start=(j == 0), stop=(j == 3))
                v_sb = p_v.tile([L, H, DH], BF16, tag="v")
                nc.vector.tensor_copy(out=v_sb,
                                      in_=pv.rearrange("p (h d) -> p h d", h=H))
                vq[bl] = v_sb
                if bl >= 1:
                    emit_attn(bl - 1)
            emit_attn(GB - 1)
        # --- output projection out = attnT.T @ Wo + bo_eff ---
        n_chunks = (T + 127) // 128 if stage >= 10 else 0
        for c in range(n_chunks):
            w = min(128, T - c * 128)
            po = ps_pp.tile([128, D], F32, tag="pp")
            for j in range(4):
                nc.tensor.matmul(po[0:w], attnT[:, j, c * 128:c * 128 + w],
                                 Wo_sb[:, j, :], start=(j == 0), stop=(j == 3))
            osb = p_osb.tile([128, D], F32, tag="osb")
            nc.vector.tensor_add(osb[0:w], po[0:w], BO_sb[0:w])
            nc.sync.dma_start(out=out_d[c * 128:c * 128 + w, :], in_=osb[0:w])

    nc.compile()
    return nc


def _get_nc():
    if "nc" not in _CACHE:
        _CACHE["nc"] = _build()
    return _CACHE["nc"]


def _prep_in_maps(inputs):
    f32 = lambda a: np.ascontiguousarray(np.asarray(a, dtype=np.float32))
    bf16 = lambda a: np.ascontiguousarray(
        np.asarray(a, dtype=np.float32).astype(ml_dtypes.bfloat16))
    shared = {k: f32(inputs[k]) for k in ("bq", "bk", "bv", "bo",
                                          "sf_b1", "sf_b2")}
    shared.update({k: bf16(inputs[k]) for k in
                   ("Wq", "Wk", "Wv", "Wo", "sf_w1", "sf_w2")})
    xT = {}
    for key, name in (("query", "xqT"), ("key", "xkT"), ("value", "xvT")):
        # [B, L, D] -> [D, B, L], feature-major (layout change only)
        xT[name] = np.asarray(inputs[key], dtype=np.float32).astype(
            ml_dtypes.bfloat16).transpose(2, 0, 1)
    in_maps = []
    for c in range(N_CORES):
        m = dict(shared)
        for name in ("xqT", "xkT", "xvT"):
            m[name] = np.ascontiguousarray(
                xT[name][:, c * BC:(c + 1) * BC, :]).reshape(D, T)
        in_maps.append(m)
    return in_maps


def run(inputs, trace=False):
    nc = _get_nc()
    in_maps = _prep_in_maps(inputs)
    res = bass_utils.run_bass_kernel_spmd(
        nc, in_maps, core_ids=list(range(N_CORES)), trace=trace)
    out = np.concatenate(
        [res.results[c]["out"].reshape(BC, L, D) for c in range(N_CORES)],
        axis=0)
    return out, res


def kernel(**inputs) -> np.ndarray:
    out, _ = run(inputs, trace=False)
    return out


# revision 26
# speedup vs baseline: 1.1904x; 1.0496x over previous
"""Trainium2 Bass kernel for nn_MultiHeadedAttention_25984552141341.

Computation (reference):
    q/k/v = (x @ W + b) split into 8 heads of 64
    scores = q @ k^T / 8
    scores += sf_net(scores)   (SoftmaxResNet over the key dim, 71)
    p = softmax(scores, axis=key)
    out = (p @ v merged) @ Wo + bo

Sharding: batch (512) split across 8 NeuronCores, 64 batches each.
All weights replicated. Each core runs an identical Bass program (SPMD).

Device-side layout strategy (per core):
  * activations live feature-on-partition / token-on-free ("transposed"
    layout, xT = [D, B*L]); the host passes query/key/value pre-transposed
    AND pre-cast to bf16 (the matmuls run in bf16 anyway), halving HBM
    traffic vs f32.
  * qT, kT produced as [dout, tok] (transposed) by matmuls with the weight
    tiles as the stationary operand; v produced per-batch in natural
    [tok, dout] layout (needed as the moving operand of p @ v).
  * scores are computed transposed: S' = [k, q] = kT_h.T @ qT_h, which is
    exactly the layout the sf-net matmuls want (contraction over k).
  * sf-net + softmax are PHASE-BATCHED per group of 8 batches to amortize
    the ACT LUT table reloads (gelu and exp live in different ACT table
    sets; each switch costs ~1.3-2.7us):
      phase A (per batch): S' -> Ssb copy -> h1 -> gelu -> h2 accumulated
        back into the S' PSUM -> S2 copied to SBUF f32 (frees the PSUM).
      phase B: all 8 exps back-to-back (one exp-table load per group),
        interleaved with the 8 v-projections so the PE stays busy (and
        warm) while the scalar engine runs the exps.
      phase C (per batch): attention matmuls + softmax normalization +
        PE transpose of the attn rows into attnT [D, tok].
    This gives 2 table loads per group (16 total) instead of 2 per pair
    of batches (64 total).
  * attention: one matmul per head with stationary E'_h = exp(scores2)
    and moving [v_h | 1] producing [q, dh] plus the softmax denominator;
    normalization by per-partition (per-q) reciprocal on vector engine.
  * attn rows are transposed back per batch with the PE transpose and
    assembled into attnT [D, tok] feeding the output projection.
  * biases: bq/bk folded into the PSUM->SBUF copies (scalar engine bias),
    bv folded into the output bias (softmax rows sum to 1, so
    p @ (v + 1 bv^T) = p @ v + 1 bv^T), bo_eff = bo + bv @ Wo computed
    on device once.

All matmuls run in bf16 (fp32 PSUM accumulation); measured end-to-end
scale-relative error vs the fp32 reference ~4e-3.
"""

import contextlib
import os

import numpy as np
import ml_dtypes

import concourse.bass as bass
import concourse.mybir as mybir
import concourse.tile as tile
from concourse import bacc
from concourse import bass_utils

F32 = mybir.dt.float32
BF16 = mybir.dt.bfloat16
AF = mybir.ActivationFunctionType

N_CORES = 8
B, L, D, H = 512, 71, 512, 8
DH = D // H  # 64
FF = 128  # sf_net hidden
BC = B // N_CORES  # 64 batches per core
T = BC * L  # 4544 tokens per core
GB = 8  # batches per group
G = BC // GB  # 8 groups
GT = GB * L  # 568 tokens per group
HALF = GT // 2  # 284

_CACHE = {}


def _build():
    stage = int(os.environ.get("KSTAGE", "99"))
    nc = bacc.Bacc("TRN2", target_bir_lowering=False, debug=False,
                   num_devices=N_CORES)

    xqT = nc.dram_tensor("xqT", [D, T], BF16, kind="ExternalInput").ap()
    xkT = nc.dram_tensor("xkT", [D, T], BF16, kind="ExternalInput").ap()
    xvT = nc.dram_tensor("xvT", [D, T], BF16, kind="ExternalInput").ap()
    Wq = nc.dram_tensor("Wq", [D, D], BF16, kind="ExternalInput").ap()
    Wk = nc.dram_tensor("Wk", [D, D], BF16, kind="ExternalInput").ap()
    Wv = nc.dram_tensor("Wv", [D, D], BF16, kind="ExternalInput").ap()
    Wo = nc.dram_tensor("Wo", [D, D], BF16, kind="ExternalInput").ap()
    bq = nc.dram_tensor("bq", [D], F32, kind="ExternalInput").ap()
    bk = nc.dram_tensor("bk", [D], F32, kind="ExternalInput").ap()
    bv = nc.dram_tensor("bv", [D], F32, kind="ExternalInput").ap()
    bo = nc.dram_tensor("bo", [D], F32, kind="ExternalInput").ap()
    w1 = nc.dram_tensor("sf_w1", [L, FF], BF16, kind="ExternalInput").ap()
    b1 = nc.dram_tensor("sf_b1", [FF], F32, kind="ExternalInput").ap()
    w2 = nc.dram_tensor("sf_w2", [FF, L], BF16, kind="ExternalInput").ap()
    b2 = nc.dram_tensor("sf_b2", [L], F32, kind="ExternalInput").ap()
    out_d = nc.dram_tensor("out", [T, D], F32, kind="ExternalOutput").ap()

    with tile.TileContext(nc) as tc, contextlib.ExitStack() as ctx:
        singles = ctx.enter_context(tc.tile_pool(name="singles", bufs=1))
        p_xt = ctx.enter_context(tc.tile_pool(name="xt", bufs=3))
        p_qk = ctx.enter_context(tc.tile_pool(name="qk", bufs=2))
        p_v = ctx.enter_context(tc.tile_pool(name="v", bufs=GB))
        p_ssb = ctx.enter_context(tc.tile_pool(name="ssb", bufs=2))
        p_s2 = ctx.enter_context(tc.tile_pool(name="s2", bufs=GB + 1))
        p_esb = ctx.enter_context(tc.tile_pool(name="esb", bufs=GB))
        p_gel = ctx.enter_context(tc.tile_pool(name="gel", bufs=2))
        p_rec = ctx.enter_context(tc.tile_pool(name="rec", bufs=2))
        p_osb = ctx.enter_context(tc.tile_pool(name="osb", bufs=3))
        p_small = ctx.enter_context(tc.tile_pool(name="small", bufs=4))
        # PSUM: 8 banks total.
        #   sc: 2 bufs x 2 banks -- S' scores (phase A); in phase C the same
        #       ring hosts [attn pa | den] pairs (pa in bank 0, den in bank 1).
        #   h1: 2 bufs x 1 bank  -- sf-net hidden halves.
        #   pp: 2 bufs x 1 bank  -- v projections (B) + output projection.
        ps_pp = ctx.enter_context(tc.tile_pool(name="pp", bufs=2, space="PSUM"))
        ps_sc = ctx.enter_context(tc.tile_pool(name="sc", bufs=2, space="PSUM"))
        ps_h1 = ctx.enter_context(tc.tile_pool(name="h1", bufs=2, space="PSUM"))

        # ---- constants / weights ----
        def w_tiles(w_ap, name):
            t = singles.tile([128, 4, D], BF16, tag=f"w_{name}")
            nc.gpsimd.dma_start(out=t, in_=w_ap.rearrange("(j p) d -> p j d", p=128))
            return t

        Wq_sb = w_tiles(Wq, "q")
        Wk_sb = w_tiles(Wk, "k")
        Wv_sb = w_tiles(Wv, "v")
        Wo_sb = w_tiles(Wo, "o")

        def b_tile(b_ap, name):
            t = singles.tile([128, 4], F32, tag=f"b_{name}")
            nc.gpsimd.dma_start(out=t, in_=b_ap.rearrange("(j p) -> p j", p=128))
            return t

        bq_sb = b_tile(bq, "q")
        bk_sb = b_tile(bk, "k")
        bq8_sb = singles.tile([128, 4], F32, tag="bq8")
        nc.scalar.mul(bq8_sb, bq_sb, 0.125)

        w1_sb = singles.tile([L, FF], BF16, tag="w1")
        nc.gpsimd.dma_start(out=w1_sb, in_=w1)
        w2_sb = singles.tile([FF, L], BF16, tag="w2")
        nc.gpsimd.dma_start(out=w2_sb, in_=w2)
        b1_sb = singles.tile([FF, 1], F32, tag="b1")
        nc.gpsimd.dma_start(out=b1_sb, in_=b1.rearrange("(p o) -> p o", o=1))
        b2_sb = singles.tile([L, 1], F32, tag="b2")
        nc.gpsimd.dma_start(out=b2_sb, in_=b2.rearrange("(p o) -> p o", o=1))

        # all-ones stationary for the softmax-denominator matmuls: each
        # output row r of ones64.T @ E' is sum_k E'[k, :], i.e. the per-
        # column denominator replicated across 64 partitions.
        ones64 = singles.tile([L, DH], BF16, tag="ones64")
        nc.gpsimd.memset(ones64, 1.0)

        # bo_eff = bo + bv @ Wo, replicated to [128, D]
        bv_sb = singles.tile([128, 4], BF16, tag="bv")
        nc.gpsimd.dma_start(out=bv_sb, in_=bv.rearrange("(j p) -> p j", p=128))
        bo_sb = singles.tile([1, D], F32, tag="bo")
        nc.gpsimd.dma_start(out=bo_sb, in_=bo.rearrange("(o d) -> o d", o=1))
        ps_bvwo = ps_pp.tile([1, D], F32, tag="pp")
        for j in range(4):
            nc.tensor.matmul(ps_bvwo, bv_sb[:, j:j + 1], Wo_sb[:, j, :],
                             start=(j == 0), stop=(j == 3))
        boeff_row = singles.tile([1, D], F32, tag="boeffrow")
        nc.vector.tensor_add(boeff_row, ps_bvwo, bo_sb)
        ones_f32 = singles.tile([1, 128], F32, tag="ones1")
        nc.vector.memset(ones_f32, 1.0)
        ps_rep = ps_pp.tile([128, D], F32, tag="pp")
        nc.tensor.matmul(ps_rep, ones_f32, boeff_row, start=True, stop=True)
        BO_sb = singles.tile([128, D], F32, tag="BO")
        nc.vector.tensor_copy(out=BO_sb, in_=ps_rep)

        attnT = singles.tile([128, 4, T], BF16, tag="attnT")

        xq3 = xqT.rearrange("(j p) t -> p j t", p=128)
        xk3 = xkT.rearrange("(j p) t -> p j t", p=128)
        xv3 = xvT.rearrange("(j p) t -> p j t", p=128)

        # xt tiles and q/k projection results, keyed by group (the ring
        # pools bound how many live at once).
        xts = {}
        qks = {}

        def emit_dma(g):
            t0 = g * GT
            xtq = p_xt.tile([128, 4, GT], BF16, tag="xtq")
            xtk = p_xt.tile([128, 4, GT], BF16, tag="xtk")
            xtv = p_xt.tile([128, 4, GT], BF16, tag="xtv")
            nc.gpsimd.dma_start(out=xtq, in_=xq3[:, :, t0:t0 + GT])
            nc.gpsimd.dma_start(out=xtk, in_=xk3[:, :, t0:t0 + GT])
            nc.gpsimd.dma_start(out=xtv, in_=xv3[:, :, t0:t0 + GT])
            xts[g] = (xtq, xtk, xtv)

        def emit_qkproj_piece(g, piece, psum_pool, ptag):
            # piece in 0..7 -> (dt_, hf).  q/k projections (transposed
            # layout [dout, tok]).  For g >= 1 these are emitted inside the
            # previous group's BC phase (psum borrowed from the then-idle
            # h1 ring) to keep the PE array dense while the exps run.
            if piece == 0:
                qks[g] = (p_qk.tile([128, 4, GT], BF16, tag="qT", name="qT"),
                          p_qk.tile([128, 4, GT], BF16, tag="kT", name="kT"))
            qT, kT = qks[g]
            xtq, xtk, _ = xts[g]
            dt_, hf = piece // 2, piece % 2
            c0 = hf * HALF
            pq = psum_pool.tile([128, HALF], F32, tag=ptag)
            for j in range(4):
                nc.tensor.matmul(
                    pq, Wq_sb[:, j, dt_ * 128:(dt_ + 1) * 128],
                    xtq[:, j, c0:c0 + HALF],
                    start=(j == 0), stop=(j == 3))
            nc.scalar.activation(
                out=qT[:, dt_, c0:c0 + HALF], in_=pq, func=AF.Identity,
                bias=bq8_sb[:, dt_:dt_ + 1], scale=0.125)
            pk = psum_pool.tile([128, HALF], F32, tag=ptag)
            for j in range(4):
                nc.tensor.matmul(
                    pk, Wk_sb[:, j, dt_ * 128:(dt_ + 1) * 128],
                    xtk[:, j, c0:c0 + HALF],
                    start=(j == 0), stop=(j == 3))
            # bias-add on the scalar engine (Identity is in every
            # ACT table set, so no table reload) to offload the DVE
            nc.scalar.activation(
                out=kT[:, dt_, c0:c0 + HALF], in_=pk, func=AF.Identity,
                bias=bk_sb[:, dt_:dt_ + 1], scale=1.0)

        emit_dma(0)
        for p in range(8):
            emit_qkproj_piece(0, p, ps_pp, "pp")

        for g in range(G):
            if stage < 1:
                break
            qT, kT = qks.pop(g)
            xtq, xtk, xtv = xts[g]
            if g + 1 < G:
                emit_dma(g + 1)

            if stage < 3:
                continue

            # --- phase A: scores + sf-net hidden + residual, per batch ---
            # All gelus of the group run back-to-back on the scalar engine
            # (one gelu-table load per group); the post-residual scores S2
            # are copied to SBUF f32, freeing the S' PSUM banks.
            s2q = {}
            for bl in range(GB):
                tc0 = bl * L

                # --- scores S' = [k, q] ---
                # PE row groups must NOT alternate between matmuls (HW
                # wedge observed when the base partition flips 0<->64
                # inside a bank group), so heads are emitted parity-
                # grouped: bank 0 hosts even heads (qkT partition base
                # 0), bank 1 odd heads (base 64).  Head h lives at
                # column 512*(h%2) + 71*(h//2).
                S_ps = ps_sc.tile([L, 1024], F32, tag="sc")
                for i in range(2):
                    for hh in range(4):  # head 2*hh+i
                        off = 512 * i + L * hh
                        nc.tensor.matmul(
                            S_ps[:, off:off + L],
                            kT[64 * i:64 * i + 64, hh, tc0:tc0 + L],
                            qT[64 * i:64 * i + 64, hh, tc0:tc0 + L],
                            start=(hh == 0), stop=False)
                S3 = S_ps.rearrange("p (b r) -> p b r", b=2)[:, :, 0:4 * L]
                Ssb = p_ssb.tile([L, 2, 4 * L], BF16, tag="Ssb")
                nc.vector.tensor_copy(out=Ssb, in_=S3)
                Sflat = Ssb.rearrange("p b r -> p (b r)")

                # --- sf-net hidden layer + output, in half-batch (1-PSUM-
                # bank) pieces aligned to the score banks ---
                gel = p_gel.tile([FF, GT], BF16, tag="gel")
                for hf in range(2):
                    # padded to a full PSUM bank so the ring slots stay
                    # bank-aligned (matmul output cannot straddle banks)
                    h1_ps = ps_h1.tile([FF, 512], F32, tag="h1")
                    nc.tensor.matmul(h1_ps[:, 0:4 * L], w1_sb,
                                     Sflat[:, 4 * L * hf:4 * L * (hf + 1)],
                                     start=True, stop=True)
                    nc.scalar.activation(
                        out=gel[:, 4 * L * hf:4 * L * (hf + 1)],
                        in_=h1_ps[:, 0:4 * L],
                        func=AF.Gelu, bias=b1_sb, scale=1.0)
                    nc.tensor.matmul(
                        S_ps[:, 512 * hf:512 * hf + 4 * L], w2_sb,
                        gel[:, 4 * L * hf:4 * L * (hf + 1)],
                        start=False, stop=True)
                # S2 to SBUF (f32, exact) -- frees the S' PSUM pair and
                # decouples the exp phase from PSUM lifetimes.
                S2sb = p_s2.tile([L, 2, 4 * L], F32, tag="S2")
                nc.vector.tensor_copy(out=S2sb, in_=S3)
                s2q[bl] = S2sb
                last_gel = gel

            # The Tile scheduler orders each engine's queue by estimated
            # readiness, not emission order, so without a real dependency
            # it interleaves the exps between the gelus and walrus emits an
            # ACT table load per switch (~1.3us each).  Force every exp
            # after the group's last gelu by routing the exp bias through a
            # copy that reads (a scrap of) the last gel tile: b2g = 0*gel+b2.
            b2g = p_small.tile([L, 1], F32, tag="b2g")
            nc.vector.scalar_tensor_tensor(
                out=b2g, in0=last_gel[0:L, 0:1], scalar=0.0, in1=b2_sb,
                op0=mybir.AluOpType.mult, op1=mybir.AluOpType.add)

            # --- phase BC: softmax exp + v projections + attention ---
            # The 8 exps run back-to-back on the scalar engine (one
            # exp-table load per group).  The v projections AND the
            # attention of the previous batch run on the PE in the same
            # window: together they keep the PE dense enough that the HAM
            # clock gate never re-throttles it to half rate (a ~10us K=4/8
            # window per group was measured with exp/attn as separate
            # phases).
            #
            # Attention is computed directly in attnT layout:
            # attnT_h = v_h.T @ E'_h needs no transposes: stationary v_h
            # [k, dh], moving E'_h [k, q] -> out [dh, q].  Head pairs
            # (2*dt, 2*dt+1) land at partition rows 0-63 / 64-127 (PE
            # column tiling) of the dt-th 128-row attnT tile.  The
            # denominators ride as a second matmul with an all-ones
            # stationary: den[r, (hh q)] = sum_k E'[k, (hh q)] for every
            # r, i.e. the reciprocal multiplier pre-replicated across
            # partitions.  pa lives in bank 0 and den in bank 1 of one
            # 2-bank "sc"-ring allocation.  The den matmuls go FIRST so
            # the DVE reciprocal runs concurrently with the attn matmuls.
            Ef = {}
            vq = {}

            def emit_attn(bl):
                bi = g * GB + bl
                v_sb = vq.pop(bl)
                Eflat = Ef.pop(bl)
                sc_t = ps_sc.tile([128, 1024], F32, tag="sc")
                pa = sc_t[:, 0:4 * L].rearrange("p (d q) -> p d q", d=4)
                den = sc_t[:, 512:512 + 4 * L].rearrange(
                    "p (d q) -> p d q", d=4)
                nc.tensor.matmul(den[0:64], ones64, Eflat[:, 0:4 * L],
                                 start=True, stop=True, tile_position=(0, 0))
                nc.tensor.matmul(den[64:128], ones64, Eflat[:, 4 * L:8 * L],
                                 start=True, stop=True, tile_position=(0, 64))
                for h in range(H):
                    rowb = 64 * (h % 2)
                    off_e = 4 * L * (h % 2) + L * (h // 2)
                    nc.tensor.matmul(
                        pa[rowb:rowb + DH, h // 2, :],
                        v_sb[:, h, :], Eflat[:, off_e:off_e + L],
                        start=True, stop=True, tile_position=(0, rowb))
                recipd = p_rec.tile([128, 4, L], F32, tag="recipd")
                nc.vector.reciprocal_approx_fast(out=recipd, in_=den)
                nc.vector.tensor_mul(
                    attnT[:, :, bi * L:(bi + 1) * L], pa, recipd)

            for bl in range(GB):
                E_sb = p_esb.tile([L, 2, 4 * L], BF16, tag="E")
                nc.scalar.activation(out=E_sb, in_=s2q[bl], func=AF.Exp,
                                     bias=b2g, scale=1.0)
                Ef[bl] = E_sb.rearrange("p b r -> p (b r)")

                tc0 = bl * L
                pv = ps_pp.tile([L, D], F32, tag="pp")
                for j in range(4):
                    nc.tensor.matmul(pv, xtv[:, j, tc0:tc0 + L],
                                     Wv_sb[:, j, :],
                                     start=(j == 0), stop=(j == 3))
                v_sb = p_v.tile([L, H, DH], BF16, tag="v")
                nc.vector.tensor_copy(out=v_sb,
                                      in_=pv.rearrange("p (h d) -> p h d", h=H))
                vq[bl] = v_sb
                # next group's q/k projection piece: dense PE-array work
                # that keeps the HAM clock gate warm through this phase
                # (the attn matmuls alone are LDWEIGHTS-bound).  Its PSUM
                # comes from the h1 ring, idle outside phase A.
                if g + 1 < G:
                    emit_qkproj_piece(g + 1, bl, ps_h1, "h1")
                if bl >= 1:
                    emit_attn(bl - 1)
            emit_attn(GB - 1)
        # --- output projection out = attnT.T @ Wo + bo_eff ---
        n_chunks = (T + 127) // 128 if stage >= 10 else 0
        for c in range(n_chunks):
            w = min(128, T - c * 128)
            po = ps_pp.tile([128, D], F32, tag="pp")
            for j in range(4):
                nc.tensor.matmul(po[0:w], attnT[:, j, c * 128:c * 128 + w],
                                 Wo_sb[:, j, :], start=(j == 0), stop=(j == 3))
            osb = p_osb.tile([128, D], F32, tag="osb")
            nc.vector.tensor_add(osb[0:w], po[0:w], BO_sb[0:w])
            nc.sync.dma_start(out=out_d[c * 128:c * 128 + w, :], in_=osb[0:w])

    nc.compile()
    return nc


def _get_nc():
    if "nc" not in _CACHE:
        _CACHE["nc"] = _build()
    return _CACHE["nc"]


def _prep_in_maps(inputs):
    f32 = lambda a: np.ascontiguousarray(np.asarray(a, dtype=np.float32))
    bf16 = lambda a: np.ascontiguousarray(
        np.asarray(a, dtype=np.float32).astype(ml_dtypes.bfloat16))
    shared = {k: f32(inputs[k]) for k in ("bq", "bk", "bv", "bo",
                                          "sf_b1", "sf_b2")}
    shared.update({k: bf16(inputs[k]) for k in
                   ("Wq", "Wk", "Wv", "Wo", "sf_w1", "sf_w2")})
    xT = {}
    for key, name in (("query", "xqT"), ("key", "xkT"), ("value", "xvT")):
        # [B, L, D] -> [D, B, L], feature-major (layout change only)
        xT[name] = np.asarray(inputs[key], dtype=np.float32).astype(
            ml_dtypes.bfloat16).transpose(2, 0, 1)
    in_maps = []
    for c in range(N_CORES):
        m = dict(shared)
        for name in ("xqT", "xkT", "xvT"):
            m[name] = np.ascontiguousarray(
                xT[name][:, c * BC:(c + 1) * BC, :]).reshape(D, T)
        in_maps.append(m)
    return in_maps


def run(inputs, trace=False):
    nc = _get_nc()
    in_maps = _prep_in_maps(inputs)
    res = bass_utils.run_bass_kernel_spmd(
        nc, in_maps, core_ids=list(range(N_CORES)), trace=trace)
    out = np.concatenate(
        [res.results[c]["out"].reshape(BC, L, D) for c in range(N_CORES)],
        axis=0)
    return out, res


def kernel(**inputs) -> np.ndarray:
    out, _ = run(inputs, trace=False)
    return out


# revision 27
# speedup vs baseline: 1.2098x; 1.0163x over previous
"""Trainium2 Bass kernel for nn_MultiHeadedAttention_25984552141341.

Computation (reference):
    q/k/v = (x @ W + b) split into 8 heads of 64
    scores = q @ k^T / 8
    scores += sf_net(scores)   (SoftmaxResNet over the key dim, 71)
    p = softmax(scores, axis=key)
    out = (p @ v merged) @ Wo + bo

Sharding: batch (512) split across 8 NeuronCores, 64 batches each.
All weights replicated. Each core runs an identical Bass program (SPMD).

Device-side layout strategy (per core):
  * activations live feature-on-partition / token-on-free ("transposed"
    layout, xT = [D, B*L]); the host passes query/key/value pre-transposed
    AND pre-cast to bf16 (the matmuls run in bf16 anyway), halving HBM
    traffic vs f32.
  * qT, kT produced as [dout, tok] (transposed) by matmuls with the weight
    tiles as the stationary operand; v produced per-batch in natural
    [tok, dout] layout (needed as the moving operand of p @ v).
  * scores are computed transposed: S' = [k, q] = kT_h.T @ qT_h, which is
    exactly the layout the sf-net matmuls want (contraction over k).
  * sf-net + softmax are PHASE-BATCHED per group of 8 batches to amortize
    the ACT LUT table reloads (gelu and exp live in different ACT table
    sets; each switch costs ~1.3-2.7us):
      phase A (per batch): S' -> Ssb copy -> h1 -> gelu -> h2 accumulated
        back into the S' PSUM -> S2 copied to SBUF f32 (frees the PSUM).
      phase B: all 8 exps back-to-back (one exp-table load per group),
        interleaved with the 8 v-projections so the PE stays busy (and
        warm) while the scalar engine runs the exps.
      phase C (per batch): attention matmuls + softmax normalization +
        PE transpose of the attn rows into attnT [D, tok].
    This gives 2 table loads per group (16 total) instead of 2 per pair
    of batches (64 total).
  * attention: one matmul per head with stationary E'_h = exp(scores2)
    and moving [v_h | 1] producing [q, dh] plus the softmax denominator;
    normalization by per-partition (per-q) reciprocal on vector engine.
  * attn rows are transposed back per batch with the PE transpose and
    assembled into attnT [D, tok] feeding the output projection.
  * biases: bq/bk folded into the PSUM->SBUF copies (scalar engine bias),
    bv folded into the output bias (softmax rows sum to 1, so
    p @ (v + 1 bv^T) = p @ v + 1 bv^T), bo_eff = bo + bv @ Wo computed
    on device once.

All matmuls run in bf16 (fp32 PSUM accumulation); measured end-to-end
scale-relative error vs the fp32 reference ~4e-3.
"""

import contextlib
import os

import numpy as np
import ml_dtypes

import concourse.bass as bass
import concourse.mybir as mybir
import concourse.tile as tile
from concourse import bacc
from concourse import bass_utils

F32 = mybir.dt.float32
BF16 = mybir.dt.bfloat16
AF = mybir.ActivationFunctionType

N_CORES = 8
B, L, D, H = 512, 71, 512, 8
DH = D // H  # 64
FF = 128  # sf_net hidden
BC = B // N_CORES  # 64 batches per core
T = BC * L  # 4544 tokens per core
GB = 8  # batches per group
G = BC // GB  # 8 groups
GT = GB * L  # 568 tokens per group
HALF = GT // 2  # 284

_CACHE = {}


def _build():
    stage = int(os.environ.get("KSTAGE", "99"))
    nc = bacc.Bacc("TRN2", target_bir_lowering=False, debug=False,
                   num_devices=N_CORES)

    xqT = nc.dram_tensor("xqT", [D, T], BF16, kind="ExternalInput").ap()
    xkT = nc.dram_tensor("xkT", [D, T], BF16, kind="ExternalInput").ap()
    xvT = nc.dram_tensor("xvT", [D, T], BF16, kind="ExternalInput").ap()
    Wq = nc.dram_tensor("Wq", [D, D], BF16, kind="ExternalInput").ap()
    Wk = nc.dram_tensor("Wk", [D, D], BF16, kind="ExternalInput").ap()
    Wv = nc.dram_tensor("Wv", [D, D], BF16, kind="ExternalInput").ap()
    Wo = nc.dram_tensor("Wo", [D, D], BF16, kind="ExternalInput").ap()
    bq = nc.dram_tensor("bq", [D], F32, kind="ExternalInput").ap()
    bk = nc.dram_tensor("bk", [D], F32, kind="ExternalInput").ap()
    bv = nc.dram_tensor("bv", [D], F32, kind="ExternalInput").ap()
    bo = nc.dram_tensor("bo", [D], F32, kind="ExternalInput").ap()
    w1 = nc.dram_tensor("sf_w1", [L, FF], BF16, kind="ExternalInput").ap()
    b1 = nc.dram_tensor("sf_b1", [FF], F32, kind="ExternalInput").ap()
    w2 = nc.dram_tensor("sf_w2", [FF, L], BF16, kind="ExternalInput").ap()
    b2 = nc.dram_tensor("sf_b2", [L], F32, kind="ExternalInput").ap()
    out_d = nc.dram_tensor("out", [T, D], F32, kind="ExternalOutput").ap()

    with tile.TileContext(nc) as tc, contextlib.ExitStack() as ctx:
        singles = ctx.enter_context(tc.tile_pool(name="singles", bufs=1))
        p_xt = ctx.enter_context(tc.tile_pool(name="xt", bufs=3))
        p_qk = ctx.enter_context(tc.tile_pool(name="qk", bufs=2))
        p_v = ctx.enter_context(tc.tile_pool(name="v", bufs=GB))
        p_ssb = ctx.enter_context(tc.tile_pool(name="ssb", bufs=2))
        p_s2 = ctx.enter_context(tc.tile_pool(name="s2", bufs=GB + 1))
        p_esb = ctx.enter_context(tc.tile_pool(name="esb", bufs=GB))
        p_gel = ctx.enter_context(tc.tile_pool(name="gel", bufs=2))
        p_rec = ctx.enter_context(tc.tile_pool(name="rec", bufs=2))
        p_osb = ctx.enter_context(tc.tile_pool(name="osb", bufs=3))
        p_small = ctx.enter_context(tc.tile_pool(name="small", bufs=4))
        # PSUM: 8 banks total.
        #   sc: 2 bufs x 2 banks -- S' scores (phase A); in phase C the same
        #       ring hosts [attn pa | den] pairs (pa in bank 0, den in bank 1).
        #   h1: 2 bufs x 1 bank  -- sf-net hidden halves.
        #   pp: 2 bufs x 1 bank  -- v projections (B) + output projection.
        ps_pp = ctx.enter_context(tc.tile_pool(name="pp", bufs=2, space="PSUM"))
        ps_sc = ctx.enter_context(tc.tile_pool(name="sc", bufs=2, space="PSUM"))
        ps_h1 = ctx.enter_context(tc.tile_pool(name="h1", bufs=2, space="PSUM"))

        # ---- constants / weights ----
        def w_tiles(w_ap, name):
            t = singles.tile([128, 4, D], BF16, tag=f"w_{name}")
            nc.gpsimd.dma_start(out=t, in_=w_ap.rearrange("(j p) d -> p j d", p=128))
            return t

        Wq_sb = w_tiles(Wq, "q")
        Wk_sb = w_tiles(Wk, "k")
        Wv_sb = w_tiles(Wv, "v")
        Wo_sb = w_tiles(Wo, "o")

        def b_tile(b_ap, name):
            t = singles.tile([128, 4], F32, tag=f"b_{name}")
            nc.gpsimd.dma_start(out=t, in_=b_ap.rearrange("(j p) -> p j", p=128))
            return t

        bq_sb = b_tile(bq, "q")
        bk_sb = b_tile(bk, "k")
        bq8_sb = singles.tile([128, 4], F32, tag="bq8")
        nc.scalar.mul(bq8_sb, bq_sb, 0.125)

        w1_sb = singles.tile([L, FF], BF16, tag="w1")
        nc.gpsimd.dma_start(out=w1_sb, in_=w1)
        w2_sb = singles.tile([FF, L], BF16, tag="w2")
        nc.gpsimd.dma_start(out=w2_sb, in_=w2)
        b1_sb = singles.tile([FF, 1], F32, tag="b1")
        nc.gpsimd.dma_start(out=b1_sb, in_=b1.rearrange("(p o) -> p o", o=1))
        b2_sb = singles.tile([L, 1], F32, tag="b2")
        nc.gpsimd.dma_start(out=b2_sb, in_=b2.rearrange("(p o) -> p o", o=1))

        # all-ones stationary for the softmax-denominator matmuls: each
        # output row r of ones64.T @ E' is sum_k E'[k, :], i.e. the per-
        # column denominator replicated across 64 partitions.
        ones64 = singles.tile([L, DH], BF16, tag="ones64")
        nc.gpsimd.memset(ones64, 1.0)

        # bo_eff = bo + bv @ Wo, replicated to [128, D]
        bv_sb = singles.tile([128, 4], BF16, tag="bv")
        nc.gpsimd.dma_start(out=bv_sb, in_=bv.rearrange("(j p) -> p j", p=128))
        bo_sb = singles.tile([1, D], F32, tag="bo")
        nc.gpsimd.dma_start(out=bo_sb, in_=bo.rearrange("(o d) -> o d", o=1))
        ps_bvwo = ps_pp.tile([1, D], F32, tag="pp")
        for j in range(4):
            nc.tensor.matmul(ps_bvwo, bv_sb[:, j:j + 1], Wo_sb[:, j, :],
                             start=(j == 0), stop=(j == 3))
        boeff_row = singles.tile([1, D], F32, tag="boeffrow")
        nc.vector.tensor_add(boeff_row, ps_bvwo, bo_sb)
        ones_f32 = singles.tile([1, 128], F32, tag="ones1")
        nc.vector.memset(ones_f32, 1.0)
        ps_rep = ps_pp.tile([128, D], F32, tag="pp")
        nc.tensor.matmul(ps_rep, ones_f32, boeff_row, start=True, stop=True)
        BO_sb = singles.tile([128, D], F32, tag="BO")
        nc.vector.tensor_copy(out=BO_sb, in_=ps_rep)

        attnT = singles.tile([128, 4, T], BF16, tag="attnT")

        xq3 = xqT.rearrange("(j p) t -> p j t", p=128)
        xk3 = xkT.rearrange("(j p) t -> p j t", p=128)
        xv3 = xvT.rearrange("(j p) t -> p j t", p=128)

        # xt tiles and q/k projection results, keyed by group (the ring
        # pools bound how many live at once).
        xts = {}
        qks = {}

        def emit_dma(g):
            t0 = g * GT
            xtq = p_xt.tile([128, 4, GT], BF16, tag="xtq")
            xtk = p_xt.tile([128, 4, GT], BF16, tag="xtk")
            xtv = p_xt.tile([128, 4, GT], BF16, tag="xtv")
            nc.gpsimd.dma_start(out=xtq, in_=xq3[:, :, t0:t0 + GT])
            nc.gpsimd.dma_start(out=xtk, in_=xk3[:, :, t0:t0 + GT])
            nc.gpsimd.dma_start(out=xtv, in_=xv3[:, :, t0:t0 + GT])
            xts[g] = (xtq, xtk, xtv)

        def emit_qkproj_piece(g, piece, psum_pool, ptag):
            # piece in 0..7 -> (dt_, hf).  q/k projections (transposed
            # layout [dout, tok]).  For g >= 1 these are emitted inside the
            # previous group's BC phase (psum borrowed from the then-idle
            # h1 ring) to keep the PE array dense while the exps run.
            if piece == 0:
                qks[g] = (p_qk.tile([128, 4, GT], BF16, tag="qT", name="qT"),
                          p_qk.tile([128, 4, GT], BF16, tag="kT", name="kT"))
            qT, kT = qks[g]
            xtq, xtk, _ = xts[g]
            dt_, hf = piece // 2, piece % 2
            c0 = hf * HALF
            pq = psum_pool.tile([128, HALF], F32, tag=ptag)
            for j in range(4):
                nc.tensor.matmul(
                    pq, Wq_sb[:, j, dt_ * 128:(dt_ + 1) * 128],
                    xtq[:, j, c0:c0 + HALF],
                    start=(j == 0), stop=(j == 3))
            nc.scalar.activation(
                out=qT[:, dt_, c0:c0 + HALF], in_=pq, func=AF.Identity,
                bias=bq8_sb[:, dt_:dt_ + 1], scale=0.125)
            pk = psum_pool.tile([128, HALF], F32, tag=ptag)
            for j in range(4):
                nc.tensor.matmul(
                    pk, Wk_sb[:, j, dt_ * 128:(dt_ + 1) * 128],
                    xtk[:, j, c0:c0 + HALF],
                    start=(j == 0), stop=(j == 3))
            # bias-add on the scalar engine (Identity is in every
            # ACT table set, so no table reload) to offload the DVE
            nc.scalar.activation(
                out=kT[:, dt_, c0:c0 + HALF], in_=pk, func=AF.Identity,
                bias=bk_sb[:, dt_:dt_ + 1], scale=1.0)

        emit_dma(0)
        for p in range(8):
            emit_qkproj_piece(0, p, ps_pp, "pp")

        for g in range(G):
            if stage < 1:
                break
            qT, kT = qks.pop(g)
            xtq, xtk, xtv = xts[g]
            if g + 1 < G:
                emit_dma(g + 1)

            if stage < 3:
                continue

            # --- phase A: scores + sf-net hidden + residual, per batch ---
            # All gelus of the group run back-to-back on the scalar engine
            # (one gelu-table load per group); the post-residual scores S2
            # are copied to SBUF f32, freeing the S' PSUM banks.
            s2q = {}
            for bl in range(GB):
                tc0 = bl * L

                # --- scores S' = [k, q] ---
                # PE row groups must NOT alternate between matmuls (HW
                # wedge observed when the base partition flips 0<->64
                # inside a bank group), so heads are emitted parity-
                # grouped: bank 0 hosts even heads (qkT partition base
                # 0), bank 1 odd heads (base 64).  Head h lives at
                # column 512*(h%2) + 71*(h//2).
                S_ps = ps_sc.tile([L, 1024], F32, tag="sc")
                for i in range(2):
                    for hh in range(4):  # head 2*hh+i
                        off = 512 * i + L * hh
                        nc.tensor.matmul(
                            S_ps[:, off:off + L],
                            kT[64 * i:64 * i + 64, hh, tc0:tc0 + L],
                            qT[64 * i:64 * i + 64, hh, tc0:tc0 + L],
                            start=(hh == 0), stop=False)
                S3 = S_ps.rearrange("p (b r) -> p b r", b=2)[:, :, 0:4 * L]
                Ssb = p_ssb.tile([L, 2, 4 * L], BF16, tag="Ssb")
                nc.vector.tensor_copy(out=Ssb, in_=S3)
                Sflat = Ssb.rearrange("p b r -> p (b r)")

                # --- sf-net hidden layer + output, in half-batch (1-PSUM-
                # bank) pieces aligned to the score banks ---
                gel = p_gel.tile([FF, GT], BF16, tag="gel")
                for hf in range(2):
                    # padded to a full PSUM bank so the ring slots stay
                    # bank-aligned (matmul output cannot straddle banks)
                    h1_ps = ps_h1.tile([FF, 512], F32, tag="h1")
                    nc.tensor.matmul(h1_ps[:, 0:4 * L], w1_sb,
                                     Sflat[:, 4 * L * hf:4 * L * (hf + 1)],
                                     start=True, stop=True)
                    nc.scalar.activation(
                        out=gel[:, 4 * L * hf:4 * L * (hf + 1)],
                        in_=h1_ps[:, 0:4 * L],
                        func=AF.Gelu, bias=b1_sb, scale=1.0)
                    nc.tensor.matmul(
                        S_ps[:, 512 * hf:512 * hf + 4 * L], w2_sb,
                        gel[:, 4 * L * hf:4 * L * (hf + 1)],
                        start=False, stop=True)
                # S2 to SBUF (f32, exact) -- frees the S' PSUM pair and
                # decouples the exp phase from PSUM lifetimes.
                S2sb = p_s2.tile([L, 2, 4 * L], F32, tag="S2")
                # alternate the copy between the vector and scalar engines:
                # phase A is otherwise DVE-paced (Ssb + S2 ~1.4us/batch vs
                # ~1.2us of PE work) while the scalar engine has slack.
                if bl % 2 == 0:
                    nc.vector.tensor_copy(out=S2sb, in_=S3)
                else:
                    nc.scalar.activation(out=S2sb, in_=S3, func=AF.Identity,
                                         bias=0.0, scale=1.0)
                s2q[bl] = S2sb
                last_gel = gel

            # The Tile scheduler orders each engine's queue by estimated
            # readiness, not emission order, so without a real dependency
            # it interleaves the exps between the gelus and walrus emits an
            # ACT table load per switch (~1.3us each).  Force every exp
            # after the group's last gelu by routing the exp bias through a
            # copy that reads (a scrap of) the last gel tile: b2g = 0*gel+b2.
            b2g = p_small.tile([L, 1], F32, tag="b2g")
            nc.vector.scalar_tensor_tensor(
                out=b2g, in0=last_gel[0:L, 0:1], scalar=0.0, in1=b2_sb,
                op0=mybir.AluOpType.mult, op1=mybir.AluOpType.add)

            # --- phase BC: softmax exp + v projections + attention ---
            # The 8 exps run back-to-back on the scalar engine (one
            # exp-table load per group).  The v projections AND the
            # attention of the previous batch run on the PE in the same
            # window: together they keep the PE dense enough that the HAM
            # clock gate never re-throttles it to half rate (a ~10us K=4/8
            # window per group was measured with exp/attn as separate
            # phases).
            #
            # Attention is computed directly in attnT layout:
            # attnT_h = v_h.T @ E'_h needs no transposes: stationary v_h
            # [k, dh], moving E'_h [k, q] -> out [dh, q].  Head pairs
            # (2*dt, 2*dt+1) land at partition rows 0-63 / 64-127 (PE
            # column tiling) of the dt-th 128-row attnT tile.  The
            # denominators ride as a second matmul with an all-ones
            # stationary: den[r, (hh q)] = sum_k E'[k, (hh q)] for every
            # r, i.e. the reciprocal multiplier pre-replicated across
            # partitions.  pa lives in bank 0 and den in bank 1 of one
            # 2-bank "sc"-ring allocation.  The den matmuls go FIRST so
            # the DVE reciprocal runs concurrently with the attn matmuls.
            Ef = {}
            vq = {}

            def emit_attn(bl):
                bi = g * GB + bl
                v_sb = vq.pop(bl)
                Eflat = Ef.pop(bl)
                sc_t = ps_sc.tile([128, 1024], F32, tag="sc")
                pa = sc_t[:, 0:4 * L].rearrange("p (d q) -> p d q", d=4)
                den = sc_t[:, 512:512 + 4 * L].rearrange(
                    "p (d q) -> p d q", d=4)
                nc.tensor.matmul(den[0:64], ones64, Eflat[:, 0:4 * L],
                                 start=True, stop=True, tile_position=(0, 0))
                nc.tensor.matmul(den[64:128], ones64, Eflat[:, 4 * L:8 * L],
                                 start=True, stop=True, tile_position=(0, 64))
                for h in range(H):
                    rowb = 64 * (h % 2)
                    off_e = 4 * L * (h % 2) + L * (h // 2)
                    nc.tensor.matmul(
                        pa[rowb:rowb + DH, h // 2, :],
                        v_sb[:, h, :], Eflat[:, off_e:off_e + L],
                        start=True, stop=True, tile_position=(0, rowb))
                recipd = p_rec.tile([128, 4, L], F32, tag="recipd")
                nc.vector.reciprocal_approx_fast(out=recipd, in_=den)
                nc.vector.tensor_mul(
                    attnT[:, :, bi * L:(bi + 1) * L], pa, recipd)

            for bl in range(GB):
                E_sb = p_esb.tile([L, 2, 4 * L], BF16, tag="E")
                nc.scalar.activation(out=E_sb, in_=s2q[bl], func=AF.Exp,
                                     bias=b2g, scale=1.0)
                Ef[bl] = E_sb.rearrange("p b r -> p (b r)")

                tc0 = bl * L
                pv = ps_pp.tile([L, D], F32, tag="pp")
                for j in range(4):
                    nc.tensor.matmul(pv, xtv[:, j, tc0:tc0 + L],
                                     Wv_sb[:, j, :],
                                     start=(j == 0), stop=(j == 3))
                v_sb = p_v.tile([L, H, DH], BF16, tag="v")
                nc.vector.tensor_copy(out=v_sb,
                                      in_=pv.rearrange("p (h d) -> p h d", h=H))
                vq[bl] = v_sb
                # next group's q/k projection piece: dense PE-array work
                # that keeps the HAM clock gate warm through this phase
                # (the attn matmuls alone are LDWEIGHTS-bound).  Its PSUM
                # comes from the h1 ring, idle outside phase A.
                if g + 1 < G:
                    emit_qkproj_piece(g + 1, bl, ps_h1, "h1")
                if bl >= 1:
                    emit_attn(bl - 1)
            emit_attn(GB - 1)
        # --- output projection out = attnT.T @ Wo + bo_eff ---
        n_chunks = (T + 127) // 128 if stage >= 10 else 0
        for c in range(n_chunks):
            w = min(128, T - c * 128)
            po = ps_pp.tile([128, D], F32, tag="pp")
            for j in range(4):
                nc.tensor.matmul(po[0:w], attnT[:, j, c * 128:c * 128 + w],
                                 Wo_sb[:, j, :], start=(j == 0), stop=(j == 3))
            osb = p_osb.tile([128, D], F32, tag="osb")
            nc.vector.tensor_add(osb[0:w], po[0:w], BO_sb[0:w])
            nc.sync.dma_start(out=out_d[c * 128:c * 128 + w, :], in_=osb[0:w])

    nc.compile()
    return nc


def _get_nc():
    if "nc" not in _CACHE:
        _CACHE["nc"] = _build()
    return _CACHE["nc"]


def _prep_in_maps(inputs):
    f32 = lambda a: np.ascontiguousarray(np.asarray(a, dtype=np.float32))
    bf16 = lambda a: np.ascontiguousarray(
        np.asarray(a, dtype=np.float32).astype(ml_dtypes.bfloat16))
    shared = {k: f32(inputs[k]) for k in ("bq", "bk", "bv", "bo",
                                          "sf_b1", "sf_b2")}
    shared.update({k: bf16(inputs[k]) for k in
                   ("Wq", "Wk", "Wv", "Wo", "sf_w1", "sf_w2")})
    xT = {}
    for key, name in (("query", "xqT"), ("key", "xkT"), ("value", "xvT")):
        # [B, L, D] -> [D, B, L], feature-major (layout change only)
        xT[name] = np.asarray(inputs[key], dtype=np.float32).astype(
            ml_dtypes.bfloat16).transpose(2, 0, 1)
    in_maps = []
    for c in range(N_CORES):
        m = dict(shared)
        for name in ("xqT", "xkT", "xvT"):
            m[name] = np.ascontiguousarray(
                xT[name][:, c * BC:(c + 1) * BC, :]).reshape(D, T)
        in_maps.append(m)
    return in_maps


def run(inputs, trace=False):
    nc = _get_nc()
    in_maps = _prep_in_maps(inputs)
    res = bass_utils.run_bass_kernel_spmd(
        nc, in_maps, core_ids=list(range(N_CORES)), trace=trace)
    out = np.concatenate(
        [res.results[c]["out"].reshape(BC, L, D) for c in range(N_CORES)],
        axis=0)
    return out, res


def kernel(**inputs) -> np.ndarray:
    out, _ = run(inputs, trace=False)
    return out
